# revision 4
# baseline (speedup 1.0000x reference)
"""Trainium2 Bass kernel for nn_AddMaskHead (ROI mask head: bilinear pool + concat + conv3x3 + BN + ReLU).

Self-contained: hardcodes shapes B=2, N=256 (512 boxes), C=256, H=96, W=128, P=14.
Shards data-parallel over the 512 boxes across 8 NeuronCores (64 boxes/core; each
core's boxes all come from a single image, so each core only needs its image's
features).

Conv strategy: 1-D Winograd F(2,3) along x. The 3x3 conv is computed as 4
x-positions (u) x 3 row taps (dy) instead of 9 taps x (2x the output columns),
cutting PE streaming 1.5x. The ROI pooling is folded into the conv's crops half
via separable interpolation matrices (my (x) mxw), with the Winograd x-transform
applied analytically to the x-side factor.
"""

import sys, os, types

sys.path.insert(0, "/opt/trn_rl_repo")

import numpy as np
import concourse.bass as bass
import concourse.mybir as mybir
import concourse.tile as tile
from concourse import bacc
from concourse.masks import make_identity

F32 = mybir.dt.float32
BF16 = mybir.dt.bfloat16
I32 = mybir.dt.int32
ALU = mybir.AluOpType
AF = mybir.ActivationFunctionType

N_CORES = 8
NB = 64            # boxes per core
BATCH = 8          # boxes per inner batch
NBATCH = NB // BATCH
HB = 4             # boxes per PSUM half
P = 14             # pooler resolution
C = 256            # channels
H, W = 96, 128     # feature map
PQ = P * P         # 196
Q1 = PQ - 128      # 68 (q-chunk 1 size)


def _axis_static(in_s, out_s=P):
    # mirrors reference._resize_bilinear axis() in exact f32 arithmetic
    s = (np.arange(out_s, dtype=np.float32) + np.float32(0.5)) * np.float32(in_s / out_s) - np.float32(0.5)
    s = np.maximum(s, np.float32(0.0))
    i0 = np.minimum(np.floor(s).astype(np.int32), in_s - 1)
    i1 = np.minimum(i0 + 1, in_s - 1)
    w = (s - i0.astype(np.float32)).astype(np.float32)
    return i0, i1, w


YS0, YS1, WYS = _axis_static(H)
XS0, XS1, WXS = _axis_static(W)


def _consts_p():
    # per-partition constants: [128, 4] = (yv_q0, xv_q0, yv_q1, xv_q1); -1 pads
    arr = np.full((128, 4), -1.0, dtype=np.float32)
    for p in range(128):
        arr[p, 0] = (p // P)
        arr[p, 1] = (p % P)
    for p in range(Q1):
        q = 128 + p
        arr[p, 2] = (q // P)
        arr[p, 3] = (q % P)
    return arr


def _consts_f():
    # free-dim constants (broadcast to all partitions on device):
    # [0:14] jc = arange(14)+0.5; [14:28] wys; [28:42] 1-wys
    arr = np.zeros((1, 48), dtype=np.float32)
    arr[0, 0:14] = np.arange(P, dtype=np.float32) + np.float32(0.5)
    arr[0, 14:28] = WYS
    arr[0, 28:42] = np.float32(1.0) - WYS
    return arr


def build_kernel():
    nc = bacc.Bacc(None)

    feat = nc.declare_dram_parameter("features", [C, H, W], F32, isOutput=False)
    boxes = nc.declare_dram_parameter("boxes", [NB, 4], F32, isOutput=False)
    mask = nc.declare_dram_parameter("mask", [NB, C, P, P], F32, isOutput=False)
    wt_d = nc.declare_dram_parameter("wt", [128, 4, 12, 256], BF16, isOutput=False)
    epi_d = nc.declare_dram_parameter("epi", [128, 5, 2], F32, isOutput=False)
    cp_d = nc.declare_dram_parameter("consts_p", [128, 4], F32, isOutput=False)
    cf_d = nc.declare_dram_parameter("consts_f", [1, 48], F32, isOutput=False)
    out_d = nc.declare_dram_parameter("out", [NB, C, P, P], F32, isOutput=True)

    mask_v = mask.rearrange("n (ch cp) i j -> cp ch n (i j)", cp=128)
    out_v = out_d.rearrange("n (oh op) i j -> op oh n (i j)", op=128)
    feat_v = feat.rearrange("(ch cp) h w -> cp ch h w", cp=128)

    RC14 = float(np.float32(1.0) / np.float32(P))

    with tile.TileContext(nc) as tc:
        with tc.tile_pool(name="persist", bufs=1) as pp:
            # ---------- persistent tiles ----------
            # x-winograd-transformed weights [ci_par, (2 mask ci_hi | 2 crop ci_hi), u*3+dy, o]
            Wt = pp.tile([128, 4, 12, 256], BF16, tag="Wt")
            # UC[qc][q, u*3+dy, o] = sum_ci cf[ci, q] * Ww_crop[u,dy][ci, o]
            UC = [pp.tile([128, 12, 256], BF16, tag=f"UC{qc}", name=f"UC{qc}") for qc in range(2)]
            # x-transformed mask features [ci_par, ci_hi, n, r(16 pad rows), u*7+tx]
            XWb = [pp.tile([128, 2, BATCH, 16, 28], BF16, tag=f"xw{i}", name=f"xw{i}")
                   for i in range(2)]
            # x-transformed interp frames per q-chunk (double-buffered)
            GWb = [[pp.tile([128, BATCH, 16, 28], BF16, tag=f"gw{qc}{i}", name=f"gw{qc}{i}")
                    for i in range(2)] for qc in range(2)]
            cpt = pp.tile([128, 4], F32, tag="cpt")
            cft = pp.tile([128, 48], F32, tag="cft")
            epi = pp.tile([128, 5, 2], F32, tag="epi")
            scale_e = pp.tile([128, 2], F32, tag="scale_e")
            bias_e = pp.tile([128, 2], F32, tag="bias_e")
            # per-box interpolation data (ping-pong slots): [128, 2, BATCH, 14] per axis
            Y0 = pp.tile([128, 2, BATCH, P], F32, tag="Y0")
            Y1 = pp.tile([128, 2, BATCH, P], F32, tag="Y1")
            WY = pp.tile([128, 2, BATCH, P], F32, tag="WY")
            OWY = pp.tile([128, 2, BATCH, P], F32, tag="OWY")
            X0 = pp.tile([128, 2, BATCH, P], F32, tag="X0")
            X1 = pp.tile([128, 2, BATCH, P], F32, tag="X1")
            WX = pp.tile([128, 2, BATCH, P], F32, tag="WX")
            OWX = pp.tile([128, 2, BATCH, P], F32, tag="OWX")
            # box-math temps
            bxb = pp.tile([128, NB, 4], F32, tag="bxb")
            abx = pp.tile([128, BATCH, 4], F32, tag="abx")
            bm_i4 = pp.tile([128, BATCH, 4], I32, tag="bm_i4")
            bm_f4 = pp.tile([128, BATCH, 4], F32, tag="bm_f4")
            bm_s = pp.tile([128, BATCH, P], F32, tag="bm_s")
            bm_f = pp.tile([128, BATCH, P], F32, tag="bm_f")
            bm_i = pp.tile([128, BATCH, P], I32, tag="bm_i")
            bm_a = pp.tile([128, BATCH, P], F32, tag="bm_a")
            bm_d = pp.tile([128, BATCH], F32, tag="bm_d")
            bm_n = pp.tile([128, BATCH], F32, tag="bm_n")
            bm_q = pp.tile([128, BATCH], F32, tag="bm_q")
            bm_h = pp.tile([128, BATCH], F32, tag="bm_h")
            cfv = pp.tile([128, 2, P, P], F32, tag="cfv")
            cfb = pp.tile([128, 2, P, P], BF16, tag="cfb")

            jc_b = cft[:, 0:14]

            def g_build(tpool, slot, sfx):
                """x-winograd interp frames GW[q, n, r16, u*7+tx] for one batch
                (ping-pong slot) from box data in `slot`."""
                for qc in range(2):
                    shb = [128, BATCH, P]
                    yv = cpt[:, 2 * qc : 2 * qc + 1, None].to_broadcast(shb)
                    xv = cpt[:, 2 * qc + 1 : 2 * qc + 2, None].to_broadcast(shb)
                    my = tpool.tile([128, BATCH, P], F32, tag=f"my{qc}{sfx}", name=f"my{qc}{sfx}")
                    mx = tpool.tile([128, BATCH, P], F32, tag=f"mx{qc}{sfx}", name=f"mx{qc}{sfx}")
                    cmp = tpool.tile([128, BATCH, P], F32, tag=f"cmp{qc}{sfx}", name=f"cmp{qc}{sfx}")
                    mxw = tpool.tile([128, BATCH, 28], F32, tag=f"mxw{qc}{sfx}", name=f"mxw{qc}{sfx}")
                    bsl = (slice(None), slot, slice(None), slice(None))
                    nc.vector.tensor_tensor(my[:], Y0[bsl], yv, ALU.is_equal)
                    nc.vector.tensor_mul(my[:], my[:], OWY[bsl])
                    nc.vector.tensor_tensor(cmp[:], Y1[bsl], yv, ALU.is_equal)
                    nc.vector.tensor_mul(cmp[:], cmp[:], WY[bsl])
                    nc.vector.tensor_add(my[:], my[:], cmp[:])
                    nc.vector.tensor_tensor(mx[:], X0[bsl], xv, ALU.is_equal)
                    nc.vector.tensor_mul(mx[:], mx[:], OWX[bsl])
                    nc.vector.tensor_tensor(cmp[:], X1[bsl], xv, ALU.is_equal)
                    nc.vector.tensor_mul(cmp[:], cmp[:], WX[bsl])
                    nc.vector.tensor_add(mx[:], mx[:], cmp[:])
                    # x-winograd transform of mx (pad col c in 1..14 <-> mx j=c-1)
                    nc.vector.tensor_sub(mxw[:, :, 1:7], mx[:, :, 1:12:2], mx[:, :, 3:14:2])
                    nc.vector.tensor_scalar_mul(mxw[:, :, 0:1], mx[:, :, 1:2], -1.0)
                    nc.vector.tensor_add(mxw[:, :, 7:14], mx[:, :, 0:14:2], mx[:, :, 1:14:2])
                    nc.vector.tensor_sub(mxw[:, :, 14:21], mx[:, :, 1:14:2], mx[:, :, 0:14:2])
                    nc.vector.tensor_sub(mxw[:, :, 21:27], mx[:, :, 0:12:2], mx[:, :, 2:14:2])
                    nc.vector.tensor_copy(mxw[:, :, 27:28], mx[:, :, 12:13])
                    # GW[:, n, 1:15, :] = my (x) mxw  (rows 0,15 stay zero)
                    GW = GWb[qc][slot]
                    shg = [128, BATCH, P, 28]
                    nc.vector.tensor_tensor(GW[:, :, 1:15, :],
                                            my[:, :, :, None].to_broadcast(shg),
                                            mxw[:, :, None, :].to_broadcast(shg), ALU.mult)

            def xw_build(mst, slot):
                """x-winograd transform of mask features into XWb[slot]."""
                XW = XWb[slot]
                v = mst[:].rearrange("p c n (i j) -> p c n i j", j=P)
                for ci in range(2):
                    xw = XW[:, ci, :, 1:15, :]
                    vi = v[:, ci]
                    nc.vector.tensor_add(xw[:, :, :, 7:14], vi[:, :, :, 0:14:2], vi[:, :, :, 1:14:2])
                    nc.vector.tensor_sub(xw[:, :, :, 14:21], vi[:, :, :, 1:14:2], vi[:, :, :, 0:14:2])
                    nc.vector.tensor_sub(xw[:, :, :, 1:7], vi[:, :, :, 1:12:2], vi[:, :, :, 3:14:2])
                    nc.vector.tensor_scalar_mul(xw[:, :, :, 0:1], vi[:, :, :, 1:2], -1.0)
                    nc.vector.tensor_sub(xw[:, :, :, 21:27], vi[:, :, :, 0:12:2], vi[:, :, :, 2:14:2])
                    nc.vector.tensor_copy(xw[:, :, :, 27:28], vi[:, :, :, 12:13])

            def box_math(n0, slot):
                """fill per-axis index/weight arrays for boxes [n0, n0+BATCH) into slot"""
                nn = BATCH
                ns = slice(n0, n0 + nn)
                t, fr, ti = abx[:], bm_f4[:], bm_i4[:]
                nc.vector.tensor_scalar_mul(t[:], bxb[:, ns], 0.125)
                nc.vector.tensor_copy(ti[:], t[:])
                nc.vector.tensor_copy(fr[:], ti[:])
                nc.vector.tensor_tensor(ti[:].bitcast(F32), fr[:], t[:], ALU.is_gt)
                nc.vector.tensor_sub(t[:], fr[:], ti[:].bitcast(F32))
                d, nlt, beq, adj = bm_d[:], bm_n[:], bm_q[:], bm_h[:]
                for ax in range(2):  # 0: x (cols 0,2), 1: y (cols 1,3)
                    a_io, b_io = t[:, :, ax], t[:, :, 2 + ax]
                    nc.vector.tensor_sub(d[:], b_io, a_io)
                    nc.vector.tensor_scalar(nlt[:], d[:], 1.0, None, ALU.is_lt)
                    nc.vector.tensor_scalar(beq[:], b_io, float(P), None, ALU.is_equal)
                    nc.vector.tensor_mul(adj[:], nlt[:], beq[:])
                    nc.vector.tensor_sub(a_io, a_io, adj[:])
                    nc.vector.tensor_add(b_io, b_io, nlt[:])
                    nc.vector.tensor_sub(b_io, b_io, adj[:])
                nwid, him1 = bm_d[:], bm_n[:]
                s, frs, si, i0c = bm_s[:], bm_f[:], bm_i[:], bm_a[:]
                sh3 = [128, nn, P]
                for ax, (I0, I1, Wf, OWf) in enumerate(
                    [(X0, X1, WX, OWX), (Y0, Y1, WY, OWY)]
                ):
                    ssl = (slice(None), slot)
                    lo_b = t[:, :, ax][:, :, None].to_broadcast(sh3)
                    nc.vector.tensor_sub(nwid[:], t[:, :, 2 + ax], t[:, :, ax])
                    nc.vector.tensor_scalar_sub(him1[:], nwid[:], 1.0)
                    h_b = him1[:, :, None].to_broadcast(sh3)
                    nc.vector.tensor_tensor(s[:], nwid[:, :, None].to_broadcast(sh3),
                                            jc_b[:, None, :].to_broadcast(sh3), ALU.mult)
                    nc.vector.tensor_scalar(s[:], s[:], RC14, -0.5, ALU.mult, ALU.add)
                    nc.vector.tensor_scalar(s[:], s[:], 0.0, None, ALU.max)
                    nc.vector.tensor_copy(si[:], s[:])
                    nc.vector.tensor_copy(frs[:], si[:])
                    nc.vector.tensor_tensor(si[:].bitcast(F32), frs[:], s[:], ALU.is_gt)
                    nc.vector.tensor_sub(i0c[:], frs[:], si[:].bitcast(F32))
                    nc.vector.tensor_tensor(i0c[:], i0c[:], h_b, ALU.min)
                    nc.vector.tensor_sub(Wf[ssl], s[:], i0c[:])
                    nc.vector.tensor_scalar(OWf[ssl], Wf[ssl], -1.0, 1.0, ALU.mult, ALU.add)
                    nc.vector.tensor_add(I0[ssl], i0c[:], lo_b)
                    nc.vector.tensor_scalar_add(i0c[:], i0c[:], 1.0)
                    nc.vector.tensor_tensor(i0c[:], i0c[:], h_b, ALU.min)
                    nc.vector.tensor_add(I1[ssl], i0c[:], lo_b)

            # ---------- phase 0 ----------
            with tc.tile_pool(name="ph0", bufs=1) as p0, \
                 tc.tile_pool(name="ps0", bufs=1, space="PSUM") as ps0:

                # --- tiny DMAs first on SP (bx1 gates box math), then weights
                ones1 = p0.tile([1, 128], F32, tag="ones1")
                nc.gpsimd.memset(ones1[:], 1.0)
                bx1 = p0.tile([1, NB * 4], F32, tag="bx1")
                nc.sync.dma_start(bx1[:], boxes.rearrange("n f -> (n f)")[None, :])
                cf1 = p0.tile([1, 48], F32, tag="cf1")
                nc.sync.dma_start(cf1[:], cf_d[:])
                nc.sync.dma_start(cpt[:], cp_d[:])

                # --- weights: bf16, host-transformed; mask chunks first
                nc.sync.dma_start(Wt[:, 0:1].rearrange("p a b c -> p (a b c)"),
                                  wt_d[:, 0:1].rearrange("p a b c -> p (a b c)"))

                # --- mask batch 0 ch0 prefetch (gates the first conv matmuls)
                mst0 = p0.tile([128, 2, BATCH, PQ], F32, tag="mst0")
                nc.sync.dma_start(mst0[:, 0], mask_v[:, 0, 0:BATCH])
                nc.sync.dma_start(Wt[:, 1:2].rearrange("p a b c -> p (a b c)"),
                                  wt_d[:, 1:2].rearrange("p a b c -> p (a b c)"))

                # --- broadcasts via K=1 matmul with ones (PE is idle here)
                psb = ps0.tile([128, 256], F32, tag="psb")
                nc.tensor.matmul(psb[:], ones1[:], bx1[:])
                nc.scalar.copy(bxb[:].rearrange("p n f -> p (n f)"), psb[:])
                psf = ps0.tile([128, 48], F32, tag="psf")
                nc.tensor.matmul(psf[:], ones1[:], cf1[:])
                nc.scalar.copy(cft[:], psf[:])

                # --- feature rows: YS1[i] == YS0[i]+1 always, so load row pairs.
                #     YS0 is piecewise-affine (stride-7 runs) -> few strided DMAs
                assert (YS1 == YS0 + 1).all()
                runs = []  # (i_start, count, step)
                rs = 0
                for i in range(1, P + 1):
                    if i == P or (i - rs >= 2 and YS0[i] - YS0[i - 1] != YS0[rs + 1] - YS0[rs]):
                        step = int(YS0[rs + 1] - YS0[rs]) if i - rs >= 2 else 1
                        runs.append((rs, i - rs, step))
                        rs = i
                R01 = p0.tile([128, 2, P, 2, W], F32, tag="R01")
                for ch in range(2):
                    for (i0r, cnt, step) in runs:
                        base = int(YS0[i0r])
                        for r in range(2):  # r = 0: YS0 rows, r = 1: YS1 rows
                            nc.sync.dma_start(
                                R01[:, ch, i0r : i0r + cnt, r],
                                feat_v[:, ch, base + r : base + r + (cnt - 1) * step + 1 : step])

                # --- mask batch 0 ch1, then crop-half weight chunks (for UC build)
                nc.sync.dma_start(mst0[:, 1], mask_v[:, 1, 0:BATCH])
                nc.sync.dma_start(Wt[:, 2:4].rearrange("p a b c -> p (a b c)"),
                                  wt_d[:, 2:4].rearrange("p a b c -> p (a b c)"))

                # --- one-time zeroing of pad rows r=0,15 (gpsimd; interiors are
                #     rewritten every batch)
                for i in range(2):
                    nc.gpsimd.memset(XWb[i][:, :, :, 0, :], 0.0)
                    nc.gpsimd.memset(XWb[i][:, :, :, 15, :], 0.0)
                for qc in range(2):
                    for i in range(2):
                        nc.gpsimd.memset(GWb[qc][i][:, :, 0, :], 0.0)
                        nc.gpsimd.memset(GWb[qc][i][:, :, 15, :], 0.0)

                # --- batch-0 operand production on DVE: XW first (unblocks mask
                #     matmuls), then box math + G frames (crops wait on UC anyway)
                xw_build(mst0, 0)
                box_math(0, 0)
                g_build(p0, 0, "b0")

                # --- concat-features (cf): x-lerp on narrow row pairs, then y-lerp
                cfx = p0.tile([128, 2, P, 2, P], F32, tag="cfx")  # (ch, i, r, j)
                tmpx = p0.tile([128, 2, P, 2], F32, tag="tmpx")
                for j in range(P):
                    nc.vector.tensor_scalar_mul(cfx[:, :, :, :, j], R01[:, :, :, :, int(XS0[j])],
                                                float(np.float32(1.0) - WXS[j]))
                    nc.vector.tensor_scalar_mul(tmpx[:], R01[:, :, :, :, int(XS1[j])], float(WXS[j]))
                    nc.vector.tensor_add(cfx[:, :, :, :, j], cfx[:, :, :, :, j], tmpx[:])
                tmpy = p0.tile([128, 2, P, P], F32, tag="tmpy")
                shc = [128, 2, P, P]
                nc.vector.tensor_tensor(cfv[:], cfx[:, :, :, 0, :],
                                        cft[:, None, 28:42, None].to_broadcast(shc), ALU.mult)
                nc.vector.tensor_tensor(tmpy[:], cfx[:, :, :, 1, :],
                                        cft[:, None, 14:28, None].to_broadcast(shc), ALU.mult)
                nc.vector.tensor_add(cfv[:], cfv[:], tmpy[:])
                nc.vector.tensor_copy(cfb[:], cfv[:])

                # --- gpsimd ucode warmup (first tensor_tensor pays ~6us IRAM load)
                warm = p0.tile([128, 8], F32, tag="warm")
                nc.gpsimd.memset(warm[:], 0.0)
                nc.gpsimd.tensor_add(warm[:, 0:4], warm[:, 0:4], warm[:, 4:8])
                nc.gpsimd.tensor_sub(warm[:, 0:4], warm[:, 0:4], warm[:, 4:8])

                # --- epilogue scalars
                nc.sync.dma_start(epi[:].rearrange("p a b -> p (a b)"),
                                  epi_d.rearrange("p a b -> p (a b)"))
                tmp_e = p0.tile([128, 2], F32, tag="tmp_e")
                eps_t = p0.tile([128, 1], F32, tag="eps_t")
                nc.vector.memset(eps_t[:], 1e-5)
                nc.scalar.activation(tmp_e[:], epi[:, 4, :], AF.Sqrt, bias=eps_t[:], scale=1.0)
                nc.vector.reciprocal(scale_e[:], tmp_e[:])
                nc.vector.tensor_mul(scale_e[:], scale_e[:], epi[:, 1, :])
                nc.vector.tensor_sub(bias_e[:], epi[:, 0, :], epi[:, 3, :])
                nc.vector.tensor_mul(bias_e[:], bias_e[:], scale_e[:])
                nc.vector.tensor_add(bias_e[:], bias_e[:], epi[:, 2, :])

            # ---------- main loop ----------
            with tc.tile_pool(name="loop", bufs=2) as lp, \
                 tc.tile_pool(name="gpool", bufs=2) as gp, \
                 tc.tile_pool(name="psc", bufs=2, space="PSUM") as psc, \
                 tc.tile_pool(name="psv", bufs=6, space="PSUM") as psv:

                cfv_f = cfb[:].rearrange("p c i j -> p c (i j)")

                def emit_mask(Mt, XW, oc, ns):
                    for u in range(4):
                        for ci in range(2):
                            for dy in range(3):
                                first = (ci == 0 and dy == 0)
                                lhsT = Wt[:, ci, u * 3 + dy, oc * 128 : oc * 128 + 128]
                                rhs = XW[:, ci, ns, dy : dy + P, u * 7 : u * 7 + 7]
                                nc.tensor.matmul(Mt[u][:], lhsT, rhs,
                                                 start=first, stop=False)

                def emit_crops(Mt, GWs, oc, ns):
                    for u in range(4):
                        for qc in range(2):
                            qn = 128 if qc == 0 else Q1
                            for dy in range(3):
                                last = (qc == 1 and dy == 2)
                                lhsT = UC[qc][:qn, u * 3 + dy, oc * 128 : oc * 128 + 128]
                                rhs = GWs[qc][:qn, ns, dy : dy + P, u * 7 : u * 7 + 7]
                                nc.tensor.matmul(Mt[u][:], lhsT, rhs,
                                                 start=False, stop=last)

                def emit_uc_build():
                    # UC[qc][q, u*3+dy, o] = sum_ci cfb[ci, q] * Wt[ci, 2+cc, udy, o]
                    for qc in range(2):
                        qn = 128 if qc == 0 else Q1
                        qs = slice(qc * 128, qc * 128 + qn)
                        for udy in range(12):
                            psU = psc.tile([128, 256], F32, tag="ups",
                                           name=f"ups{qc}_{udy}")
                            for cc in range(2):
                                nc.tensor.matmul(psU[:qn], cfv_f[:, cc, qs],
                                                 Wt[:, 2 + cc, udy, :],
                                                 start=(cc == 0), stop=(cc == 1))
                            nc.scalar.copy(UC[qc][:qn, udy, :], psU[:qn])

                for b in range(NBATCH):
                    n0 = b * BATCH
                    slot = b % 2
                    XW = XWb[slot]
                    GWs = [GWb[0][slot], GWb[1][slot]]
                    # operands for batch b (XW/GW/box data) were produced during
                    # batch b-1 (batch 0's in phase 0)

                    ost = lp.tile([128, 2, BATCH, PQ], F32, tag="ost")
                    ost_v = ost

                    for oc in range(2):
                        Yt = lp.tile([128, BATCH, P, P], F32, tag="Yt", name=f"Y_{b}_{oc}")
                        Yv = Yt[:].rearrange("p n i j -> p n (i j)")
                        for half in range(2):
                            ns = slice(half * HB, half * HB + HB)
                            Mt = [psv.tile([128, HB, P, 7], F32, tag="M",
                                           name=f"M_{b}_{oc}_{half}_{u}")
                                  for u in range(4)]
                            emit_mask(Mt, XW, oc, ns)
                            if b == 0 and oc == 0 and half == 0:
                                # UC build: after first mask matmuls so the PE has
                                # work while the cf chain finishes on DVE
                                emit_uc_build()
                            emit_crops(Mt, GWs, oc, ns)
                            # PSUM -> SBUF on the scalar engine (frees banks fast,
                            # decoupled from DVE bulk work)
                            Mc = lp.tile([128, 4, HB, P, 7], F32, tag="Mc",
                                         name=f"Mc_{b}_{oc}_{half}")
                            for u in range(4):
                                nc.scalar.copy(Mc[:, u], Mt[u][:])
                            # inverse x-transform: Y[...,0::2] = M0+M1+M2,
                            # Y[...,1::2] = M1-M2-M3; temps on gpsimd, Y on DVE
                            tI = gp.tile([128, 2, HB, P, 7], F32, tag="tI",
                                         name=f"tI_{b}_{oc}_{half}")
                            nc.gpsimd.tensor_add(tI[:, 0], Mc[:, 0], Mc[:, 1])
                            nc.gpsimd.tensor_sub(tI[:, 1], Mc[:, 1], Mc[:, 2])
                            nc.vector.tensor_add(Yt[:, ns, :, 0::2], tI[:, 0], Mc[:, 2])
                            nc.vector.tensor_sub(Yt[:, ns, :, 1::2], tI[:, 1], Mc[:, 3])
                            # BN + ReLU + store
                            nc.scalar.activation(
                                ost_v[:, oc, ns], Yv[:, ns],
                                AF.Relu, bias=bias_e[:, oc : oc + 1],
                                scale=scale_e[:, oc : oc + 1],
                            )
                            nc.sync.dma_start(
                                out_v[:, oc, n0 + half * HB : n0 + half * HB + HB],
                                ost_v[:, oc, ns])
                        # next batch's operands, emitted mid-batch so the DVE
                        # produces them while the PE runs this batch's passes
                        if oc == 0 and b + 1 < NBATCH:
                            nslot = (b + 1) % 2
                            box_math(n0 + BATCH, nslot)
                            mstn = lp.tile([128, 2, BATCH, PQ], F32, tag="mst",
                                           name=f"mst{b + 1}")
                            for ch in range(2):
                                nc.sync.dma_start(mstn[:, ch],
                                                  mask_v[:, ch, n0 + BATCH : n0 + 2 * BATCH])
                            xw_build(mstn, nslot)
                            g_build(gp, nslot, "")

    nc.compile()
    return nc


# ---------------------------------------------------------------------------
# host-side sharding / unsharding
# ---------------------------------------------------------------------------

def _prep_in_maps(features, proposal_boxes, mask_features, conv_w, conv_b,
                  bn_gamma, bn_beta, bn_mean, bn_var):
    features = np.asarray(features, dtype=np.float32)
    proposal_boxes = np.asarray(proposal_boxes, dtype=np.float32)
    mask_features = np.asarray(mask_features, dtype=np.float32)
    conv_w = np.asarray(conv_w, dtype=np.float32)
    # weight layout: x-winograd transform Ww[u,dy] = sum_dx G[u,dx] w[.,.,dy,dx]
    # [cout=256, cin=512, 3, 3] -> [cin_par=128, cin_hi=4, u*3+dy (12), cout=256], bf16
    import ml_dtypes
    Gm = np.array([[1, 0, 0], [.5, .5, .5], [.5, -.5, .5], [0, 0, 1]], np.float32)
    wf = conv_w.reshape(256, 4, 128, 3, 3)                     # [o, hi, par, dy, dx]
    ww = np.einsum('ud,ohpyd->phuyo', Gm, wf)                  # [par, hi, u, dy, o]
    wt = np.ascontiguousarray(ww.reshape(128, 4, 12, 256)).astype(ml_dtypes.bfloat16)
    epi = np.stack([np.asarray(x, dtype=np.float32) for x in
                    (conv_b, bn_gamma, bn_beta, bn_mean, bn_var)])  # [5, 256]
    epi = np.ascontiguousarray(epi.reshape(5, 2, 128).transpose(2, 0, 1)).astype(np.float32)
    cp = _consts_p()
    cfc = _consts_f()

    in_maps = []
    for i in range(N_CORES):
        img = i // (N_CORES // 2)
        n0 = (i * NB) % 256
        in_maps.append({
            "features": np.ascontiguousarray(features[img]),
            "boxes": np.ascontiguousarray(proposal_boxes[img, n0 : n0 + NB]),
            "mask": np.ascontiguousarray(mask_features[i * NB : (i + 1) * NB]),
            "wt": wt,
            "epi": epi,
            "consts_p": cp,
            "consts_f": cfc,
        })
    return in_maps


_NC_CACHE = {}


def _get_nc():
    if "nc" not in _NC_CACHE:
        _NC_CACHE["nc"] = build_kernel()
    return _NC_CACHE["nc"]


def _install_ntff_shim():
    """antenv.axon_hooks is missing in this image; shim it so trace=True works."""
    try:
        import antenv
        if hasattr(antenv, "axon_hooks"):
            return
        from trn_agent_boot.trn_boot import _ntff_profile_via_ctypes
        mod = types.ModuleType("antenv.axon_hooks")
        _h = [None]
        mod.set_axon_ntff_profile_hook = lambda h: _h.__setitem__(0, h)
        mod.get_axon_ntff_profile_hook = lambda: _h[0]
        sys.modules["antenv.axon_hooks"] = mod
        antenv.axon_hooks = mod
        mod.set_axon_ntff_profile_hook(_ntff_profile_via_ctypes("/opt/axon/libaxon_pjrt.so"))
    except Exception:
        pass


def run(trace=False, tmpdir=None, **inputs):
    from concourse.bass_utils import run_bass_kernel_spmd

    if trace:
        _install_ntff_shim()
    nc = _get_nc()
    in_maps = _prep_in_maps(**inputs)
    res = run_bass_kernel_spmd(nc, in_maps, core_ids=list(range(N_CORES)),
                               trace=trace, tmpdir=tmpdir)
    out = np.concatenate([np.asarray(res.results[i]["out"]) for i in range(N_CORES)], axis=0)
    return out.astype(np.float32), res


def kernel(**inputs):
    out, _ = run(trace=False, **inputs)
    return out


# revision 5
# speedup vs baseline: 1.1659x; 1.1659x over previous
"""Trainium2 Bass kernel for nn_AddMaskHead (ROI mask head: bilinear pool + concat + conv3x3 + BN + ReLU).

Self-contained: hardcodes shapes B=2, N=256 (512 boxes), C=256, H=96, W=128, P=14.
Shards data-parallel over the 512 boxes across 8 NeuronCores (64 boxes/core; each
core's boxes all come from a single image, so each core only needs its image's
features).

Conv strategy: 1-D Winograd F(2,3) along x. The 3x3 conv is computed as 4
x-positions (u) x 3 row taps (dy) instead of 9 taps x (2x the output columns),
cutting PE streaming 1.5x. The ROI pooling is folded into the conv's crops half
via separable interpolation matrices (my (x) mxw), with the Winograd x-transform
applied analytically to the x-side factor. Each weight load (LDWEIGHTS is not
hidden on trn2) is amortized over two consecutive matmuls (the two 4-box PSUM
halves); the inverse x-transform runs on gpsimd+DVE from SBUF after fast
scalar-engine PSUM evacuation.
"""

import sys, os, types

sys.path.insert(0, "/opt/trn_rl_repo")

import numpy as np
import concourse.bass as bass
import concourse.mybir as mybir
import concourse.tile as tile
from concourse import bacc
from concourse.masks import make_identity

F32 = mybir.dt.float32
BF16 = mybir.dt.bfloat16
I32 = mybir.dt.int32
ALU = mybir.AluOpType
AF = mybir.ActivationFunctionType

N_CORES = 8
NB = 64            # boxes per core
BATCH = 8          # boxes per inner batch
NBATCH = NB // BATCH
HB = 4             # boxes per PSUM half
P = 14             # pooler resolution
C = 256            # channels
H, W = 96, 128     # feature map
PQ = P * P         # 196
Q1 = PQ - 128      # 68 (q-chunk 1 size)


def _axis_static(in_s, out_s=P):
    # mirrors reference._resize_bilinear axis() in exact f32 arithmetic
    s = (np.arange(out_s, dtype=np.float32) + np.float32(0.5)) * np.float32(in_s / out_s) - np.float32(0.5)
    s = np.maximum(s, np.float32(0.0))
    i0 = np.minimum(np.floor(s).astype(np.int32), in_s - 1)
    i1 = np.minimum(i0 + 1, in_s - 1)
    w = (s - i0.astype(np.float32)).astype(np.float32)
    return i0, i1, w


YS0, YS1, WYS = _axis_static(H)
XS0, XS1, WXS = _axis_static(W)


def _consts_p():
    # per-partition constants: [128, 4] = (yv_q0, xv_q0, yv_q1, xv_q1); -1 pads
    arr = np.full((128, 4), -1.0, dtype=np.float32)
    for p in range(128):
        arr[p, 0] = (p // P)
        arr[p, 1] = (p % P)
    for p in range(Q1):
        q = 128 + p
        arr[p, 2] = (q // P)
        arr[p, 3] = (q % P)
    return arr


def _consts_f():
    # free-dim constants (broadcast to all partitions on device):
    # [0:14] jc = arange(14)+0.5; [14:28] wys; [28:42] 1-wys
    arr = np.zeros((1, 48), dtype=np.float32)
    arr[0, 0:14] = np.arange(P, dtype=np.float32) + np.float32(0.5)
    arr[0, 14:28] = WYS
    arr[0, 28:42] = np.float32(1.0) - WYS
    return arr


def build_kernel():
    nc = bacc.Bacc(None)

    feat = nc.declare_dram_parameter("features", [C, H, W], F32, isOutput=False)
    boxes = nc.declare_dram_parameter("boxes", [NB, 4], F32, isOutput=False)
    mask = nc.declare_dram_parameter("mask", [NB, C, P, P], F32, isOutput=False)
    wt_d = nc.declare_dram_parameter("wt", [128, 4, 12, 256], BF16, isOutput=False)
    epi_d = nc.declare_dram_parameter("epi", [128, 5, 2], F32, isOutput=False)
    cp_d = nc.declare_dram_parameter("consts_p", [128, 4], F32, isOutput=False)
    cf_d = nc.declare_dram_parameter("consts_f", [1, 48], F32, isOutput=False)
    out_d = nc.declare_dram_parameter("out", [NB, C, P, P], F32, isOutput=True)

    mask_v = mask.rearrange("n (ch cp) i j -> cp ch n (i j)", cp=128)
    out_v = out_d.rearrange("n (oh op) i j -> op oh n (i j)", op=128)
    feat_v = feat.rearrange("(ch cp) h w -> cp ch h w", cp=128)

    RC14 = float(np.float32(1.0) / np.float32(P))

    with tile.TileContext(nc) as tc:
        with tc.tile_pool(name="persist", bufs=1) as pp:
            # ---------- persistent tiles ----------
            # x-winograd-transformed mask-half weights [ci_par, ci_hi, u*3+dy, o]
            Wm = pp.tile([128, 2, 12, 256], BF16, tag="Wm")
            # UC[qc][q, u*3+dy, o] = sum_ci cf[ci, q] * Ww_crop[u,dy][ci, o]
            UC = [pp.tile([128, 12, 256], BF16, tag=f"UC{qc}", name=f"UC{qc}") for qc in range(2)]
            # x-transformed mask features [ci_par, ci_hi, n, r(16 pad rows), u*7+tx]
            XWb = [pp.tile([128, 2, BATCH, 16, 28], BF16, tag=f"xw{i}", name=f"xw{i}")
                   for i in range(2)]
            # x-transformed interp frames per q-chunk (double-buffered)
            GWb = [[pp.tile([128, BATCH, 16, 28], BF16, tag=f"gw{qc}{i}", name=f"gw{qc}{i}")
                    for i in range(2)] for qc in range(2)]
            cpt = pp.tile([128, 4], F32, tag="cpt")
            cft = pp.tile([128, 48], F32, tag="cft")
            epi = pp.tile([128, 5, 2], F32, tag="epi")
            scale_e = pp.tile([128, 2], F32, tag="scale_e")
            bias_e = pp.tile([128, 2], F32, tag="bias_e")
            # per-box interpolation data (ping-pong slots): [128, 2, BATCH, 14] per axis
            Y0 = pp.tile([128, 2, BATCH, P], F32, tag="Y0")
            Y1 = pp.tile([128, 2, BATCH, P], F32, tag="Y1")
            WY = pp.tile([128, 2, BATCH, P], F32, tag="WY")
            OWY = pp.tile([128, 2, BATCH, P], F32, tag="OWY")
            X0 = pp.tile([128, 2, BATCH, P], F32, tag="X0")
            X1 = pp.tile([128, 2, BATCH, P], F32, tag="X1")
            WX = pp.tile([128, 2, BATCH, P], F32, tag="WX")
            OWX = pp.tile([128, 2, BATCH, P], F32, tag="OWX")
            # box-math temps
            bxb = pp.tile([128, NB, 4], F32, tag="bxb")
            abx = pp.tile([128, BATCH, 4], F32, tag="abx")
            bm_i4 = pp.tile([128, BATCH, 4], I32, tag="bm_i4")
            bm_f4 = pp.tile([128, BATCH, 4], F32, tag="bm_f4")
            bm_s = pp.tile([128, BATCH, P], F32, tag="bm_s")
            bm_f = pp.tile([128, BATCH, P], F32, tag="bm_f")
            bm_i = pp.tile([128, BATCH, P], I32, tag="bm_i")
            bm_a = pp.tile([128, BATCH, P], F32, tag="bm_a")
            bm_d = pp.tile([128, BATCH], F32, tag="bm_d")
            bm_n = pp.tile([128, BATCH], F32, tag="bm_n")
            bm_q = pp.tile([128, BATCH], F32, tag="bm_q")
            bm_h = pp.tile([128, BATCH], F32, tag="bm_h")
            cfv = pp.tile([128, 2, P, P], F32, tag="cfv")
            cfb = pp.tile([128, 2, P, P], BF16, tag="cfb")

            jc_b = cft[:, 0:14]

            def g_build(tpool, slot, sfx):
                """x-winograd interp frames GW[q, n, r16, u*7+tx] for one batch
                (ping-pong slot) from box data in `slot`."""
                for qc in range(2):
                    shb = [128, BATCH, P]
                    yv = cpt[:, 2 * qc : 2 * qc + 1, None].to_broadcast(shb)
                    xv = cpt[:, 2 * qc + 1 : 2 * qc + 2, None].to_broadcast(shb)
                    my = tpool.tile([128, BATCH, P], F32, tag=f"my{qc}{sfx}", name=f"my{qc}{sfx}")
                    mx = tpool.tile([128, BATCH, P], F32, tag=f"mx{qc}{sfx}", name=f"mx{qc}{sfx}")
                    cmp = tpool.tile([128, BATCH, P], F32, tag=f"cmp{qc}{sfx}", name=f"cmp{qc}{sfx}")
                    mxw = tpool.tile([128, BATCH, 28], F32, tag=f"mxw{qc}{sfx}", name=f"mxw{qc}{sfx}")
                    bsl = (slice(None), slot, slice(None), slice(None))
                    nc.vector.tensor_tensor(my[:], Y0[bsl], yv, ALU.is_equal)
                    nc.vector.tensor_mul(my[:], my[:], OWY[bsl])
                    nc.vector.tensor_tensor(cmp[:], Y1[bsl], yv, ALU.is_equal)
                    nc.vector.tensor_mul(cmp[:], cmp[:], WY[bsl])
                    nc.vector.tensor_add(my[:], my[:], cmp[:])
                    nc.vector.tensor_tensor(mx[:], X0[bsl], xv, ALU.is_equal)
                    nc.vector.tensor_mul(mx[:], mx[:], OWX[bsl])
                    nc.vector.tensor_tensor(cmp[:], X1[bsl], xv, ALU.is_equal)
                    nc.vector.tensor_mul(cmp[:], cmp[:], WX[bsl])
                    nc.vector.tensor_add(mx[:], mx[:], cmp[:])
                    # x-winograd transform of mx (pad col c in 1..14 <-> mx j=c-1)
                    nc.vector.tensor_sub(mxw[:, :, 1:7], mx[:, :, 1:12:2], mx[:, :, 3:14:2])
                    nc.vector.tensor_scalar_mul(mxw[:, :, 0:1], mx[:, :, 1:2], -1.0)
                    nc.vector.tensor_add(mxw[:, :, 7:14], mx[:, :, 0:14:2], mx[:, :, 1:14:2])
                    nc.vector.tensor_sub(mxw[:, :, 14:21], mx[:, :, 1:14:2], mx[:, :, 0:14:2])
                    nc.vector.tensor_sub(mxw[:, :, 21:27], mx[:, :, 0:12:2], mx[:, :, 2:14:2])
                    nc.vector.tensor_copy(mxw[:, :, 27:28], mx[:, :, 12:13])
                    # GW[:, n, 1:15, :] = my (x) mxw  (rows 0,15 stay zero)
                    GW = GWb[qc][slot]
                    shg = [128, BATCH, P, 28]
                    nc.vector.tensor_tensor(GW[:, :, 1:15, :],
                                            my[:, :, :, None].to_broadcast(shg),
                                            mxw[:, :, None, :].to_broadcast(shg), ALU.mult)

            def xw_build(mst, slot):
                """x-winograd transform of mask features into XWb[slot]."""
                XW = XWb[slot]
                v = mst[:].rearrange("p c n (i j) -> p c n i j", j=P)
                for ci in range(2):
                    xw = XW[:, ci, :, 1:15, :]
                    vi = v[:, ci]
                    nc.vector.tensor_add(xw[:, :, :, 7:14], vi[:, :, :, 0:14:2], vi[:, :, :, 1:14:2])
                    nc.vector.tensor_sub(xw[:, :, :, 14:21], vi[:, :, :, 1:14:2], vi[:, :, :, 0:14:2])
                    nc.vector.tensor_sub(xw[:, :, :, 1:7], vi[:, :, :, 1:12:2], vi[:, :, :, 3:14:2])
                    nc.vector.tensor_scalar_mul(xw[:, :, :, 0:1], vi[:, :, :, 1:2], -1.0)
                    nc.vector.tensor_sub(xw[:, :, :, 21:27], vi[:, :, :, 0:12:2], vi[:, :, :, 2:14:2])
                    nc.vector.tensor_copy(xw[:, :, :, 27:28], vi[:, :, :, 12:13])

            def box_math(n0, slot):
                """fill per-axis index/weight arrays for boxes [n0, n0+BATCH) into slot"""
                nn = BATCH
                ns = slice(n0, n0 + nn)
                t, fr, ti = abx[:], bm_f4[:], bm_i4[:]
                nc.vector.tensor_scalar_mul(t[:], bxb[:, ns], 0.125)
                nc.vector.tensor_copy(ti[:], t[:])
                nc.vector.tensor_copy(fr[:], ti[:])
                nc.vector.tensor_tensor(ti[:].bitcast(F32), fr[:], t[:], ALU.is_gt)
                nc.vector.tensor_sub(t[:], fr[:], ti[:].bitcast(F32))
                d, nlt, beq, adj = bm_d[:], bm_n[:], bm_q[:], bm_h[:]
                for ax in range(2):  # 0: x (cols 0,2), 1: y (cols 1,3)
                    a_io, b_io = t[:, :, ax], t[:, :, 2 + ax]
                    nc.vector.tensor_sub(d[:], b_io, a_io)
                    nc.vector.tensor_scalar(nlt[:], d[:], 1.0, None, ALU.is_lt)
                    nc.vector.tensor_scalar(beq[:], b_io, float(P), None, ALU.is_equal)
                    nc.vector.tensor_mul(adj[:], nlt[:], beq[:])
                    nc.vector.tensor_sub(a_io, a_io, adj[:])
                    nc.vector.tensor_add(b_io, b_io, nlt[:])
                    nc.vector.tensor_sub(b_io, b_io, adj[:])
                nwid, him1 = bm_d[:], bm_n[:]
                s, frs, si, i0c = bm_s[:], bm_f[:], bm_i[:], bm_a[:]
                sh3 = [128, nn, P]
                for ax, (I0, I1, Wf, OWf) in enumerate(
                    [(X0, X1, WX, OWX), (Y0, Y1, WY, OWY)]
                ):
                    ssl = (slice(None), slot)
                    lo_b = t[:, :, ax][:, :, None].to_broadcast(sh3)
                    nc.vector.tensor_sub(nwid[:], t[:, :, 2 + ax], t[:, :, ax])
                    nc.vector.tensor_scalar_sub(him1[:], nwid[:], 1.0)
                    h_b = him1[:, :, None].to_broadcast(sh3)
                    nc.vector.tensor_tensor(s[:], nwid[:, :, None].to_broadcast(sh3),
                                            jc_b[:, None, :].to_broadcast(sh3), ALU.mult)
                    nc.vector.tensor_scalar(s[:], s[:], RC14, -0.5, ALU.mult, ALU.add)
                    nc.vector.tensor_scalar(s[:], s[:], 0.0, None, ALU.max)
                    nc.vector.tensor_copy(si[:], s[:])
                    nc.vector.tensor_copy(frs[:], si[:])
                    nc.vector.tensor_tensor(si[:].bitcast(F32), frs[:], s[:], ALU.is_gt)
                    nc.vector.tensor_sub(i0c[:], frs[:], si[:].bitcast(F32))
                    nc.vector.tensor_tensor(i0c[:], i0c[:], h_b, ALU.min)
                    nc.vector.tensor_sub(Wf[ssl], s[:], i0c[:])
                    nc.vector.tensor_scalar(OWf[ssl], Wf[ssl], -1.0, 1.0, ALU.mult, ALU.add)
                    nc.vector.tensor_add(I0[ssl], i0c[:], lo_b)
                    nc.vector.tensor_scalar_add(i0c[:], i0c[:], 1.0)
                    nc.vector.tensor_tensor(i0c[:], i0c[:], h_b, ALU.min)
                    nc.vector.tensor_add(I1[ssl], i0c[:], lo_b)

            # ---------- phase 0 ----------
            with tc.tile_pool(name="ph0", bufs=1) as p0, \
                 tc.tile_pool(name="ps0", bufs=1, space="PSUM") as ps0, \
                 tc.tile_pool(name="psu", bufs=2, space="PSUM") as psu0:

                # --- tiny DMAs first on SP (bx1 gates box math), then features
                ones1 = p0.tile([1, 128], F32, tag="ones1")
                nc.gpsimd.memset(ones1[:], 1.0)
                bx1 = p0.tile([1, NB * 4], F32, tag="bx1")
                nc.sync.dma_start(bx1[:], boxes.rearrange("n f -> (n f)")[None, :])
                cf1 = p0.tile([1, 48], F32, tag="cf1")
                nc.sync.dma_start(cf1[:], cf_d[:])
                nc.sync.dma_start(cpt[:], cp_d[:])

                # --- feature rows early (gate the cf -> UC chain).
                #     YS1[i] == YS0[i]+1 always, so load row pairs; YS0 is
                #     piecewise-affine (stride-7 runs) -> few strided DMAs
                assert (YS1 == YS0 + 1).all()
                runs = []  # (i_start, count, step)
                rs = 0
                for i in range(1, P + 1):
                    if i == P or (i - rs >= 2 and YS0[i] - YS0[i - 1] != YS0[rs + 1] - YS0[rs]):
                        step = int(YS0[rs + 1] - YS0[rs]) if i - rs >= 2 else 1
                        runs.append((rs, i - rs, step))
                        rs = i
                R01 = p0.tile([128, 2, P, 2, W], F32, tag="R01")
                for ch in range(2):
                    for (i0r, cnt, step) in runs:
                        base = int(YS0[i0r])
                        for r in range(2):  # r = 0: YS0 rows, r = 1: YS1 rows
                            nc.sync.dma_start(
                                R01[:, ch, i0r : i0r + cnt, r],
                                feat_v[:, ch, base + r : base + r + (cnt - 1) * step + 1 : step])

                # --- mask batch 0 + weights (crop-half weights gate UC build)
                mst0 = p0.tile([128, 2, BATCH, PQ], F32, tag="mst0")
                nc.sync.dma_start(mst0[:, 0], mask_v[:, 0, 0:BATCH])
                nc.sync.dma_start(mst0[:, 1], mask_v[:, 1, 0:BATCH])
                Wc = p0.tile([128, 2, 12, 256], BF16, tag="Wc")
                nc.sync.dma_start(Wc[:].rearrange("p a b c -> p (a b c)"),
                                  wt_d[:, 2:4].rearrange("p a b c -> p (a b c)"))
                nc.sync.dma_start(Wm[:].rearrange("p a b c -> p (a b c)"),
                                  wt_d[:, 0:2].rearrange("p a b c -> p (a b c)"))

                # --- broadcasts via K=1 matmul with ones (PE is idle here)
                psb = ps0.tile([128, 256], F32, tag="psb")
                nc.tensor.matmul(psb[:], ones1[:], bx1[:])
                nc.scalar.copy(bxb[:].rearrange("p n f -> p (n f)"), psb[:])
                psf = ps0.tile([128, 48], F32, tag="psf")
                nc.tensor.matmul(psf[:], ones1[:], cf1[:])
                nc.scalar.copy(cft[:], psf[:])

                # --- one-time zeroing of pad rows r=0,15 (gpsimd; interiors are
                #     rewritten every batch)
                for i in range(2):
                    nc.gpsimd.memset(XWb[i][:, :, :, 0, :], 0.0)
                    nc.gpsimd.memset(XWb[i][:, :, :, 15, :], 0.0)
                for qc in range(2):
                    for i in range(2):
                        nc.gpsimd.memset(GWb[qc][i][:, :, 0, :], 0.0)
                        nc.gpsimd.memset(GWb[qc][i][:, :, 15, :], 0.0)

                # --- concat-features (cf) first on DVE: gates the UC build
                cfx = p0.tile([128, 2, P, 2, P], F32, tag="cfx")  # (ch, i, r, j)
                tmpx = p0.tile([128, 2, P, 2], F32, tag="tmpx")
                for j in range(P):
                    nc.vector.tensor_scalar_mul(cfx[:, :, :, :, j], R01[:, :, :, :, int(XS0[j])],
                                                float(np.float32(1.0) - WXS[j]))
                    nc.vector.tensor_scalar_mul(tmpx[:], R01[:, :, :, :, int(XS1[j])], float(WXS[j]))
                    nc.vector.tensor_add(cfx[:, :, :, :, j], cfx[:, :, :, :, j], tmpx[:])
                tmpy = p0.tile([128, 2, P, P], F32, tag="tmpy")
                shc = [128, 2, P, P]
                nc.vector.tensor_tensor(cfv[:], cfx[:, :, :, 0, :],
                                        cft[:, None, 28:42, None].to_broadcast(shc), ALU.mult)
                nc.vector.tensor_tensor(tmpy[:], cfx[:, :, :, 1, :],
                                        cft[:, None, 14:28, None].to_broadcast(shc), ALU.mult)
                nc.vector.tensor_add(cfv[:], cfv[:], tmpy[:])
                nc.vector.tensor_copy(cfb[:], cfv[:])

                # --- batch-0 operands on DVE (after cf so UC starts earliest)
                xw_build(mst0, 0)
                box_math(0, 0)
                g_build(p0, 0, "b0")

                # --- UC build on PE (phase 0 PSUM; frees loop-phase banks):
                #     UC[qc][q, udy, o] = sum_ci cfb[ci, q] * Wc[ci_hi][ci, udy, o]
                cfv_f = cfb[:].rearrange("p c i j -> p c (i j)")
                for qc in range(2):
                    qn = 128 if qc == 0 else Q1
                    qs = slice(qc * 128, qc * 128 + qn)
                    for udy in range(12):
                        psU = psu0.tile([128, 256], F32, tag="ups",
                                        name=f"ups{qc}_{udy}")
                        for cc in range(2):
                            nc.tensor.matmul(psU[:qn], cfv_f[:, cc, qs],
                                             Wc[:, cc, udy, :],
                                             start=(cc == 0), stop=(cc == 1))
                        nc.scalar.copy(UC[qc][:qn, udy, :], psU[:qn])

                # --- gpsimd ucode warmup (first tensor_tensor pays ~6us IRAM load)
                warm = p0.tile([128, 8], F32, tag="warm")
                nc.gpsimd.memset(warm[:], 0.0)
                nc.gpsimd.tensor_add(warm[:, 0:4], warm[:, 0:4], warm[:, 4:8])
                nc.gpsimd.tensor_sub(warm[:, 0:4], warm[:, 0:4], warm[:, 4:8])

                # --- epilogue scalars
                nc.sync.dma_start(epi[:].rearrange("p a b -> p (a b)"),
                                  epi_d.rearrange("p a b -> p (a b)"))
                tmp_e = p0.tile([128, 2], F32, tag="tmp_e")
                eps_t = p0.tile([128, 1], F32, tag="eps_t")
                nc.vector.memset(eps_t[:], 1e-5)
                nc.scalar.activation(tmp_e[:], epi[:, 4, :], AF.Sqrt, bias=eps_t[:], scale=1.0)
                nc.vector.reciprocal(scale_e[:], tmp_e[:])
                nc.vector.tensor_mul(scale_e[:], scale_e[:], epi[:, 1, :])
                nc.vector.tensor_sub(bias_e[:], epi[:, 0, :], epi[:, 3, :])
                nc.vector.tensor_mul(bias_e[:], bias_e[:], scale_e[:])
                nc.vector.tensor_add(bias_e[:], bias_e[:], epi[:, 2, :])

            # ---------- main loop ----------
            with tc.tile_pool(name="loop", bufs=2) as lp, \
                 tc.tile_pool(name="gpool", bufs=2) as gp, \
                 tc.tile_pool(name="psv", bufs=8, space="PSUM") as psv:

                def emit_pass(Mt2, XW, GWs, oc):
                    # 12 weight chunks per u; each lhsT feeds both 4-box halves
                    # back-to-back so LDWEIGHTS is paid once per chunk
                    for u in range(4):
                        k = 0
                        for ci in range(2):
                            for dy in range(3):
                                lhsT = Wm[:, ci, u * 3 + dy, oc * 128 : oc * 128 + 128]
                                for h in range(2):
                                    ns = slice(h * HB, h * HB + HB)
                                    rhs = XW[:, ci, ns, dy : dy + P, u * 7 : u * 7 + 7]
                                    nc.tensor.matmul(Mt2[u][h][:], lhsT, rhs,
                                                     start=(k == 0), stop=False)
                                k += 1
                        for qc in range(2):
                            qn = 128 if qc == 0 else Q1
                            for dy in range(3):
                                lhsT = UC[qc][:qn, u * 3 + dy, oc * 128 : oc * 128 + 128]
                                for h in range(2):
                                    ns = slice(h * HB, h * HB + HB)
                                    rhs = GWs[qc][:qn, ns, dy : dy + P, u * 7 : u * 7 + 7]
                                    nc.tensor.matmul(Mt2[u][h][:], lhsT, rhs,
                                                     start=False, stop=(k == 11))
                                k += 1

                for b in range(NBATCH):
                    n0 = b * BATCH
                    slot = b % 2
                    XW = XWb[slot]
                    GWs = [GWb[0][slot], GWb[1][slot]]
                    # operands for batch b (XW/GW/box data) were produced during
                    # batch b-1 (batch 0's in phase 0)

                    ost = lp.tile([128, 2, BATCH, PQ], F32, tag="ost")
                    ost_v = ost

                    for oc in range(2):
                        Yt = lp.tile([128, BATCH, P, P], F32, tag="Yt", name=f"Y_{b}_{oc}")
                        Yv = Yt[:].rearrange("p n i j -> p n (i j)")
                        Mt2 = [[psv.tile([128, HB, P, 7], F32, tag="M",
                                         name=f"M_{b}_{oc}_{u}_{h}")
                                for h in range(2)] for u in range(4)]
                        emit_pass(Mt2, XW, GWs, oc)
                        # PSUM -> SBUF on the scalar engine (frees banks fast,
                        # decoupled from DVE bulk work)
                        Mc = lp.tile([128, 4, BATCH, P, 7], F32, tag="Mc",
                                     name=f"Mc_{b}_{oc}")
                        for u in range(4):
                            for h in range(2):
                                nc.scalar.copy(Mc[:, u, h * HB : h * HB + HB],
                                               Mt2[u][h][:])
                        # inverse x-transform: Y[...,0::2] = M0+M1+M2,
                        # Y[...,1::2] = M1-M2-M3; temps on gpsimd, Y on DVE
                        tI = gp.tile([128, 2, BATCH, P, 7], F32, tag="tI",
                                     name=f"tI_{b}_{oc}")
                        nc.gpsimd.tensor_add(tI[:, 0], Mc[:, 0], Mc[:, 1])
                        nc.gpsimd.tensor_sub(tI[:, 1], Mc[:, 1], Mc[:, 2])
                        nc.vector.tensor_add(Yt[:, :, :, 0::2], tI[:, 0], Mc[:, 2])
                        nc.vector.tensor_sub(Yt[:, :, :, 1::2], tI[:, 1], Mc[:, 3])
                        # BN + ReLU + store (split for DMA overlap)
                        for h in range(2):
                            ns = slice(h * HB, h * HB + HB)
                            nc.scalar.activation(
                                ost_v[:, oc, ns], Yv[:, ns],
                                AF.Relu, bias=bias_e[:, oc : oc + 1],
                                scale=scale_e[:, oc : oc + 1],
                            )
                            nc.sync.dma_start(
                                out_v[:, oc, n0 + h * HB : n0 + h * HB + HB],
                                ost_v[:, oc, ns])
                        # next batch's operands, emitted mid-batch so the DVE
                        # produces them while the PE runs this batch's passes
                        if oc == 0 and b + 1 < NBATCH:
                            nslot = (b + 1) % 2
                            box_math(n0 + BATCH, nslot)
                            mstn = lp.tile([128, 2, BATCH, PQ], F32, tag="mst",
                                           name=f"mst{b + 1}")
                            for ch in range(2):
                                nc.sync.dma_start(mstn[:, ch],
                                                  mask_v[:, ch, n0 + BATCH : n0 + 2 * BATCH])
                            xw_build(mstn, nslot)
                            g_build(gp, nslot, "")

    nc.compile()
    return nc


# ---------------------------------------------------------------------------
# host-side sharding / unsharding
# ---------------------------------------------------------------------------

def _prep_in_maps(features, proposal_boxes, mask_features, conv_w, conv_b,
                  bn_gamma, bn_beta, bn_mean, bn_var):
    features = np.asarray(features, dtype=np.float32)
    proposal_boxes = np.asarray(proposal_boxes, dtype=np.float32)
    mask_features = np.asarray(mask_features, dtype=np.float32)
    conv_w = np.asarray(conv_w, dtype=np.float32)
    # weight layout: x-winograd transform Ww[u,dy] = sum_dx G[u,dx] w[.,.,dy,dx]
    # [cout=256, cin=512, 3, 3] -> [cin_par=128, cin_hi=4, u*3+dy (12), cout=256], bf16
    import ml_dtypes
    Gm = np.array([[1, 0, 0], [.5, .5, .5], [.5, -.5, .5], [0, 0, 1]], np.float32)
    wf = conv_w.reshape(256, 4, 128, 3, 3)                     # [o, hi, par, dy, dx]
    ww = np.einsum('ud,ohpyd->phuyo', Gm, wf)                  # [par, hi, u, dy, o]
    wt = np.ascontiguousarray(ww.reshape(128, 4, 12, 256)).astype(ml_dtypes.bfloat16)
    epi = np.stack([np.asarray(x, dtype=np.float32) for x in
                    (conv_b, bn_gamma, bn_beta, bn_mean, bn_var)])  # [5, 256]
    epi = np.ascontiguousarray(epi.reshape(5, 2, 128).transpose(2, 0, 1)).astype(np.float32)
    cp = _consts_p()
    cfc = _consts_f()

    in_maps = []
    for i in range(N_CORES):
        img = i // (N_CORES // 2)
        n0 = (i * NB) % 256
        in_maps.append({
            "features": np.ascontiguousarray(features[img]),
            "boxes": np.ascontiguousarray(proposal_boxes[img, n0 : n0 + NB]),
            "mask": np.ascontiguousarray(mask_features[i * NB : (i + 1) * NB]),
            "wt": wt,
            "epi": epi,
            "consts_p": cp,
            "consts_f": cfc,
        })
    return in_maps


_NC_CACHE = {}


def _get_nc():
    if "nc" not in _NC_CACHE:
        _NC_CACHE["nc"] = build_kernel()
    return _NC_CACHE["nc"]


def _install_ntff_shim():
    """antenv.axon_hooks is missing in this image; shim it so trace=True works."""
    try:
        import antenv
        if hasattr(antenv, "axon_hooks"):
            return
        from trn_agent_boot.trn_boot import _ntff_profile_via_ctypes
        mod = types.ModuleType("antenv.axon_hooks")
        _h = [None]
        mod.set_axon_ntff_profile_hook = lambda h: _h.__setitem__(0, h)
        mod.get_axon_ntff_profile_hook = lambda: _h[0]
        sys.modules["antenv.axon_hooks"] = mod
        antenv.axon_hooks = mod
        mod.set_axon_ntff_profile_hook(_ntff_profile_via_ctypes("/opt/axon/libaxon_pjrt.so"))
    except Exception:
        pass


def run(trace=False, tmpdir=None, **inputs):
    from concourse.bass_utils import run_bass_kernel_spmd

    if trace:
        _install_ntff_shim()
    nc = _get_nc()
    in_maps = _prep_in_maps(**inputs)
    res = run_bass_kernel_spmd(nc, in_maps, core_ids=list(range(N_CORES)),
                               trace=trace, tmpdir=tmpdir)
    out = np.concatenate([np.asarray(res.results[i]["out"]) for i in range(N_CORES)], axis=0)
    return out.astype(np.float32), res


def kernel(**inputs):
    out, _ = run(trace=False, **inputs)
    return out


# revision 7
# speedup vs baseline: 1.1761x; 1.0087x over previous
"""Trainium2 Bass kernel for nn_AddMaskHead (ROI mask head: bilinear pool + concat + conv3x3 + BN + ReLU).

Self-contained: hardcodes shapes B=2, N=256 (512 boxes), C=256, H=96, W=128, P=14.
Shards data-parallel over the 512 boxes across 8 NeuronCores (64 boxes/core; each
core's boxes all come from a single image, so each core only needs its image's
features).

Conv strategy: 1-D Winograd F(2,3) along x. The 3x3 conv is computed as 4
x-positions (u) x 3 row taps (dy) instead of 9 taps x (2x the output columns),
cutting PE streaming 1.5x. The ROI pooling is folded into the conv's crops half
via separable interpolation matrices (my (x) mxw), with the Winograd x-transform
applied analytically to the x-side factor. Each weight load (LDWEIGHTS is not
hidden on trn2) is amortized over two consecutive matmuls (the two 4-box PSUM
halves); the inverse x-transform runs on gpsimd+DVE from SBUF after fast
scalar-engine PSUM evacuation.
"""

import sys, os, types

sys.path.insert(0, "/opt/trn_rl_repo")

import numpy as np
import concourse.bass as bass
import concourse.mybir as mybir
import concourse.tile as tile
from concourse import bacc
from concourse.masks import make_identity

F32 = mybir.dt.float32
BF16 = mybir.dt.bfloat16
I32 = mybir.dt.int32
ALU = mybir.AluOpType
AF = mybir.ActivationFunctionType

N_CORES = 8
NB = 64            # boxes per core
BATCH = 8          # boxes per inner batch
NBATCH = NB // BATCH
HB = 4             # boxes per PSUM half
P = 14             # pooler resolution
C = 256            # channels
H, W = 96, 128     # feature map
PQ = P * P         # 196
Q1 = PQ - 128      # 68 (q-chunk 1 size)


def _axis_static(in_s, out_s=P):
    # mirrors reference._resize_bilinear axis() in exact f32 arithmetic
    s = (np.arange(out_s, dtype=np.float32) + np.float32(0.5)) * np.float32(in_s / out_s) - np.float32(0.5)
    s = np.maximum(s, np.float32(0.0))
    i0 = np.minimum(np.floor(s).astype(np.int32), in_s - 1)
    i1 = np.minimum(i0 + 1, in_s - 1)
    w = (s - i0.astype(np.float32)).astype(np.float32)
    return i0, i1, w


YS0, YS1, WYS = _axis_static(H)
XS0, XS1, WXS = _axis_static(W)


def _consts_p():
    # per-partition constants: [128, 4] = (yv_q0, xv_q0, yv_q1, xv_q1); -1 pads
    arr = np.full((128, 4), -1.0, dtype=np.float32)
    for p in range(128):
        arr[p, 0] = (p // P)
        arr[p, 1] = (p % P)
    for p in range(Q1):
        q = 128 + p
        arr[p, 2] = (q // P)
        arr[p, 3] = (q % P)
    return arr


def _consts_f():
    # free-dim constants (broadcast to all partitions on device):
    # [0:14] jc = arange(14)+0.5; [14:28] wys; [28:42] 1-wys;
    # [48:62] wxs; [62:76] 1-wxs
    arr = np.zeros((1, 80), dtype=np.float32)
    arr[0, 0:14] = np.arange(P, dtype=np.float32) + np.float32(0.5)
    arr[0, 14:28] = WYS
    arr[0, 28:42] = np.float32(1.0) - WYS
    arr[0, 48:62] = WXS
    arr[0, 62:76] = np.float32(1.0) - WXS
    return arr


def build_kernel():
    nc = bacc.Bacc(None)

    feat = nc.declare_dram_parameter("features", [C, H, W], F32, isOutput=False)
    boxes = nc.declare_dram_parameter("boxes", [NB, 4], F32, isOutput=False)
    mask = nc.declare_dram_parameter("mask", [NB, C, P, P], F32, isOutput=False)
    wt_d = nc.declare_dram_parameter("wt", [128, 4, 12, 256], BF16, isOutput=False)
    epi_d = nc.declare_dram_parameter("epi", [128, 5, 2], F32, isOutput=False)
    cp_d = nc.declare_dram_parameter("consts_p", [128, 4], F32, isOutput=False)
    cf_d = nc.declare_dram_parameter("consts_f", [1, 80], F32, isOutput=False)
    out_d = nc.declare_dram_parameter("out", [NB, C, P, P], F32, isOutput=True)

    mask_v = mask.rearrange("n (ch cp) i j -> cp ch n (i j)", cp=128)
    out_v = out_d.rearrange("n (oh op) i j -> op oh n (i j)", op=128)
    feat_v = feat.rearrange("(ch cp) h w -> cp ch h w", cp=128)

    RC14 = float(np.float32(1.0) / np.float32(P))

    with tile.TileContext(nc) as tc:
        with tc.tile_pool(name="persist", bufs=1) as pp:
            # ---------- persistent tiles ----------
            # x-winograd-transformed mask-half weights [ci_par, ci_hi, u*3+dy, o]
            Wm = pp.tile([128, 2, 12, 256], BF16, tag="Wm")
            # UC[qc][q, u*3+dy, o] = sum_ci cf[ci, q] * Ww_crop[u,dy][ci, o]
            UC = [pp.tile([128, 12, 256], BF16, tag=f"UC{qc}", name=f"UC{qc}") for qc in range(2)]
            # x-transformed mask features [ci_par, ci_hi, n, r(16 pad rows), u*7+tx]
            XWb = [pp.tile([128, 2, BATCH, 16, 28], BF16, tag=f"xw{i}", name=f"xw{i}")
                   for i in range(2)]
            # x-transformed interp frames per q-chunk (double-buffered)
            GWb = [[pp.tile([128, BATCH, 16, 28], BF16, tag=f"gw{qc}{i}", name=f"gw{qc}{i}")
                    for i in range(2)] for qc in range(2)]
            cpt = pp.tile([128, 4], F32, tag="cpt")
            cft = pp.tile([128, 80], F32, tag="cft")
            epi = pp.tile([128, 5, 2], F32, tag="epi")
            scale_e = pp.tile([128, 2], F32, tag="scale_e")
            bias_e = pp.tile([128, 2], F32, tag="bias_e")
            # per-box interpolation data (ping-pong slots): [128, 2, BATCH, 14] per axis
            Y0 = pp.tile([128, 2, BATCH, P], F32, tag="Y0")
            Y1 = pp.tile([128, 2, BATCH, P], F32, tag="Y1")
            WY = pp.tile([128, 2, BATCH, P], F32, tag="WY")
            OWY = pp.tile([128, 2, BATCH, P], F32, tag="OWY")
            X0 = pp.tile([128, 2, BATCH, P], F32, tag="X0")
            X1 = pp.tile([128, 2, BATCH, P], F32, tag="X1")
            WX = pp.tile([128, 2, BATCH, P], F32, tag="WX")
            OWX = pp.tile([128, 2, BATCH, P], F32, tag="OWX")
            # box-math temps
            bxb = pp.tile([128, NB, 4], F32, tag="bxb")
            abx = pp.tile([128, BATCH, 4], F32, tag="abx")
            bm_i4 = pp.tile([128, BATCH, 4], I32, tag="bm_i4")
            bm_f4 = pp.tile([128, BATCH, 4], F32, tag="bm_f4")
            bm_s = pp.tile([128, BATCH, P], F32, tag="bm_s")
            bm_f = pp.tile([128, BATCH, P], F32, tag="bm_f")
            bm_i = pp.tile([128, BATCH, P], I32, tag="bm_i")
            bm_a = pp.tile([128, BATCH, P], F32, tag="bm_a")
            bm_d = pp.tile([128, BATCH], F32, tag="bm_d")
            bm_n = pp.tile([128, BATCH], F32, tag="bm_n")
            bm_q = pp.tile([128, BATCH], F32, tag="bm_q")
            bm_h = pp.tile([128, BATCH], F32, tag="bm_h")
            cfv = pp.tile([128, 2, P, P], F32, tag="cfv")
            cfb = pp.tile([128, 2, P, P], BF16, tag="cfb")

            jc_b = cft[:, 0:14]

            def g_build(tpool, slot, sfx):
                """x-winograd interp frames GW[q, n, r16, u*7+tx] for one batch
                (ping-pong slot) from box data in `slot`."""
                for qc in range(2):
                    shb = [128, BATCH, P]
                    yv = cpt[:, 2 * qc : 2 * qc + 1, None].to_broadcast(shb)
                    xv = cpt[:, 2 * qc + 1 : 2 * qc + 2, None].to_broadcast(shb)
                    my = tpool.tile([128, BATCH, P], F32, tag=f"my{qc}{sfx}", name=f"my{qc}{sfx}")
                    mx = tpool.tile([128, BATCH, P], F32, tag=f"mx{qc}{sfx}", name=f"mx{qc}{sfx}")
                    cmp = tpool.tile([128, BATCH, P], F32, tag=f"cmp{qc}{sfx}", name=f"cmp{qc}{sfx}")
                    mxw = tpool.tile([128, BATCH, 28], F32, tag=f"mxw{qc}{sfx}", name=f"mxw{qc}{sfx}")
                    bsl = (slice(None), slot, slice(None), slice(None))
                    nc.vector.tensor_tensor(my[:], Y0[bsl], yv, ALU.is_equal)
                    nc.vector.tensor_mul(my[:], my[:], OWY[bsl])
                    nc.vector.tensor_tensor(cmp[:], Y1[bsl], yv, ALU.is_equal)
                    nc.vector.tensor_mul(cmp[:], cmp[:], WY[bsl])
                    nc.vector.tensor_add(my[:], my[:], cmp[:])
                    nc.vector.tensor_tensor(mx[:], X0[bsl], xv, ALU.is_equal)
                    nc.vector.tensor_mul(mx[:], mx[:], OWX[bsl])
                    nc.vector.tensor_tensor(cmp[:], X1[bsl], xv, ALU.is_equal)
                    nc.vector.tensor_mul(cmp[:], cmp[:], WX[bsl])
                    nc.vector.tensor_add(mx[:], mx[:], cmp[:])
                    # x-winograd transform of mx (pad col c in 1..14 <-> mx j=c-1)
                    nc.vector.tensor_sub(mxw[:, :, 1:7], mx[:, :, 1:12:2], mx[:, :, 3:14:2])
                    nc.vector.tensor_scalar_mul(mxw[:, :, 0:1], mx[:, :, 1:2], -1.0)
                    nc.vector.tensor_add(mxw[:, :, 7:14], mx[:, :, 0:14:2], mx[:, :, 1:14:2])
                    nc.vector.tensor_sub(mxw[:, :, 14:21], mx[:, :, 1:14:2], mx[:, :, 0:14:2])
                    nc.vector.tensor_sub(mxw[:, :, 21:27], mx[:, :, 0:12:2], mx[:, :, 2:14:2])
                    nc.vector.tensor_copy(mxw[:, :, 27:28], mx[:, :, 12:13])
                    # GW[:, n, 1:15, :] = my (x) mxw  (rows 0,15 stay zero)
                    GW = GWb[qc][slot]
                    shg = [128, BATCH, P, 28]
                    nc.vector.tensor_tensor(GW[:, :, 1:15, :],
                                            my[:, :, :, None].to_broadcast(shg),
                                            mxw[:, :, None, :].to_broadcast(shg), ALU.mult)

            def xw_build(mst, slot):
                """x-winograd transform of mask features into XWb[slot]."""
                XW = XWb[slot]
                v = mst[:].rearrange("p c n (i j) -> p c n i j", j=P)
                for ci in range(2):
                    xw = XW[:, ci, :, 1:15, :]
                    vi = v[:, ci]
                    nc.vector.tensor_add(xw[:, :, :, 7:14], vi[:, :, :, 0:14:2], vi[:, :, :, 1:14:2])
                    nc.vector.tensor_sub(xw[:, :, :, 14:21], vi[:, :, :, 1:14:2], vi[:, :, :, 0:14:2])
                    nc.vector.tensor_sub(xw[:, :, :, 1:7], vi[:, :, :, 1:12:2], vi[:, :, :, 3:14:2])
                    nc.vector.tensor_scalar_mul(xw[:, :, :, 0:1], vi[:, :, :, 1:2], -1.0)
                    nc.vector.tensor_sub(xw[:, :, :, 21:27], vi[:, :, :, 0:12:2], vi[:, :, :, 2:14:2])
                    nc.vector.tensor_copy(xw[:, :, :, 27:28], vi[:, :, :, 12:13])

            def box_math(n0, slot):
                """fill per-axis index/weight arrays for boxes [n0, n0+BATCH) into slot"""
                nn = BATCH
                ns = slice(n0, n0 + nn)
                t, fr, ti = abx[:], bm_f4[:], bm_i4[:]
                nc.vector.tensor_scalar_mul(t[:], bxb[:, ns], 0.125)
                nc.vector.tensor_copy(ti[:], t[:])
                nc.vector.tensor_copy(fr[:], ti[:])
                nc.vector.tensor_tensor(ti[:].bitcast(F32), fr[:], t[:], ALU.is_gt)
                nc.vector.tensor_sub(t[:], fr[:], ti[:].bitcast(F32))
                d, nlt, beq, adj = bm_d[:], bm_n[:], bm_q[:], bm_h[:]
                for ax in range(2):  # 0: x (cols 0,2), 1: y (cols 1,3)
                    a_io, b_io = t[:, :, ax], t[:, :, 2 + ax]
                    nc.vector.tensor_sub(d[:], b_io, a_io)
                    nc.vector.tensor_scalar(nlt[:], d[:], 1.0, None, ALU.is_lt)
                    nc.vector.tensor_scalar(beq[:], b_io, float(P), None, ALU.is_equal)
                    nc.vector.tensor_mul(adj[:], nlt[:], beq[:])
                    nc.vector.tensor_sub(a_io, a_io, adj[:])
                    nc.vector.tensor_add(b_io, b_io, nlt[:])
                    nc.vector.tensor_sub(b_io, b_io, adj[:])
                nwid, him1 = bm_d[:], bm_n[:]
                s, frs, si, i0c = bm_s[:], bm_f[:], bm_i[:], bm_a[:]
                sh3 = [128, nn, P]
                for ax, (I0, I1, Wf, OWf) in enumerate(
                    [(X0, X1, WX, OWX), (Y0, Y1, WY, OWY)]
                ):
                    ssl = (slice(None), slot)
                    lo_b = t[:, :, ax][:, :, None].to_broadcast(sh3)
                    nc.vector.tensor_sub(nwid[:], t[:, :, 2 + ax], t[:, :, ax])
                    nc.vector.tensor_scalar_sub(him1[:], nwid[:], 1.0)
                    h_b = him1[:, :, None].to_broadcast(sh3)
                    nc.vector.tensor_tensor(s[:], nwid[:, :, None].to_broadcast(sh3),
                                            jc_b[:, None, :].to_broadcast(sh3), ALU.mult)
                    nc.vector.tensor_scalar(s[:], s[:], RC14, -0.5, ALU.mult, ALU.add)
                    nc.vector.tensor_scalar(s[:], s[:], 0.0, None, ALU.max)
                    nc.vector.tensor_copy(si[:], s[:])
                    nc.vector.tensor_copy(frs[:], si[:])
                    nc.vector.tensor_tensor(si[:].bitcast(F32), frs[:], s[:], ALU.is_gt)
                    nc.vector.tensor_sub(i0c[:], frs[:], si[:].bitcast(F32))
                    nc.vector.tensor_tensor(i0c[:], i0c[:], h_b, ALU.min)
                    nc.vector.tensor_sub(Wf[ssl], s[:], i0c[:])
                    nc.vector.tensor_scalar(OWf[ssl], Wf[ssl], -1.0, 1.0, ALU.mult, ALU.add)
                    nc.vector.tensor_add(I0[ssl], i0c[:], lo_b)
                    nc.vector.tensor_scalar_add(i0c[:], i0c[:], 1.0)
                    nc.vector.tensor_tensor(i0c[:], i0c[:], h_b, ALU.min)
                    nc.vector.tensor_add(I1[ssl], i0c[:], lo_b)

            # ---------- phase 0 ----------
            with tc.tile_pool(name="ph0", bufs=1) as p0, \
                 tc.tile_pool(name="ps0", bufs=1, space="PSUM") as ps0, \
                 tc.tile_pool(name="psu", bufs=2, space="PSUM") as psu0:

                # --- tiny DMAs first on SP (bx1 gates box math), then features
                ones1 = p0.tile([1, 128], F32, tag="ones1")
                nc.gpsimd.memset(ones1[:], 1.0)
                bx1 = p0.tile([1, NB * 4], F32, tag="bx1")
                nc.sync.dma_start(bx1[:], boxes.rearrange("n f -> (n f)")[None, :])
                cf1 = p0.tile([1, 80], F32, tag="cf1")
                nc.sync.dma_start(cf1[:], cf_d[:])
                nc.sync.dma_start(cpt[:], cp_d[:])

                # --- feature rows early (gate the cf -> UC chain).
                #     YS1[i] == YS0[i]+1 always, so load row pairs; YS0 is
                #     piecewise-affine (stride-7 runs) -> few strided DMAs
                assert (YS1 == YS0 + 1).all()
                runs = []  # (i_start, count, step)
                rs = 0
                for i in range(1, P + 1):
                    if i == P or (i - rs >= 2 and YS0[i] - YS0[i - 1] != YS0[rs + 1] - YS0[rs]):
                        step = int(YS0[rs + 1] - YS0[rs]) if i - rs >= 2 else 1
                        runs.append((rs, i - rs, step))
                        rs = i
                R01 = p0.tile([128, 2, P, 2, W], F32, tag="R01")
                for ch in range(2):
                    for (i0r, cnt, step) in runs:
                        base = int(YS0[i0r])
                        for r in range(2):  # r = 0: YS0 rows, r = 1: YS1 rows
                            nc.sync.dma_start(
                                R01[:, ch, i0r : i0r + cnt, r],
                                feat_v[:, ch, base + r : base + r + (cnt - 1) * step + 1 : step])

                # --- mask batch 0 + weights (crop-half weights gate UC build)
                mst0 = p0.tile([128, 2, BATCH, PQ], F32, tag="mst0")
                nc.sync.dma_start(mst0[:, 0], mask_v[:, 0, 0:BATCH])
                nc.sync.dma_start(mst0[:, 1], mask_v[:, 1, 0:BATCH])
                Wc = p0.tile([128, 2, 12, 256], BF16, tag="Wc")
                nc.sync.dma_start(Wc[:].rearrange("p a b c -> p (a b c)"),
                                  wt_d[:, 2:4].rearrange("p a b c -> p (a b c)"))
                nc.sync.dma_start(Wm[:].rearrange("p a b c -> p (a b c)"),
                                  wt_d[:, 0:2].rearrange("p a b c -> p (a b c)"))

                # --- broadcasts via K=1 matmul with ones (PE is idle here)
                psb = ps0.tile([128, 256], F32, tag="psb")
                nc.tensor.matmul(psb[:], ones1[:], bx1[:])
                nc.scalar.copy(bxb[:].rearrange("p n f -> p (n f)"), psb[:])
                psf = ps0.tile([128, 80], F32, tag="psf")
                nc.tensor.matmul(psf[:], ones1[:], cf1[:])
                nc.scalar.copy(cft[:], psf[:])

                # --- one-time zeroing of pad rows r=0,15 (gpsimd; interiors are
                #     rewritten every batch)
                for i in range(2):
                    nc.gpsimd.memset(XWb[i][:, :, :, 0, :], 0.0)
                    nc.gpsimd.memset(XWb[i][:, :, :, 15, :], 0.0)
                for qc in range(2):
                    for i in range(2):
                        nc.gpsimd.memset(GWb[qc][i][:, :, 0, :], 0.0)
                        nc.gpsimd.memset(GWb[qc][i][:, :, 15, :], 0.0)

                # --- batch-0 mask operand first on DVE (gates b0 conv pass)
                xw_build(mst0, 0)

                # --- concat-features (cf): x-lerp via stride-9 runs of XS0,
                #     per-j weights from free-dim consts; then y-lerp
                cfx = p0.tile([128, 2, P, 2, P], F32, tag="cfx")  # (ch, i, r, j)
                tmpx = p0.tile([128, 2, P, 2, P], F32, tag="tmpx")
                cfx_v = cfx[:].rearrange("p c i r j -> p c (i r) j")
                tmpx_v = tmpx[:].rearrange("p c i r j -> p c (i r) j")
                R01_v = R01[:].rearrange("p c i r w -> p c (i r) w")
                runs_x = []
                rsx = 0
                for j in range(1, P + 1):
                    if j == P or (j - rsx >= 2 and XS0[j] - XS0[j - 1] != XS0[rsx + 1] - XS0[rsx]):
                        stepx = int(XS0[rsx + 1] - XS0[rsx]) if j - rsx >= 2 else 1
                        runs_x.append((rsx, j - rsx, stepx))
                        rsx = j
                assert (XS1 == XS0 + 1).all()
                for (j0, L, stepx) in runs_x:
                    a = int(XS0[j0])
                    shL = [128, 2, 2 * P, L]
                    sl0 = slice(a, a + (L - 1) * stepx + 1, stepx)
                    sl1 = slice(a + 1, a + 1 + (L - 1) * stepx + 1, stepx)
                    w1b = cft[:, None, None, 62 + j0 : 62 + j0 + L].to_broadcast(shL)
                    wb = cft[:, None, None, 48 + j0 : 48 + j0 + L].to_broadcast(shL)
                    nc.vector.tensor_tensor(cfx_v[:, :, :, j0 : j0 + L],
                                            R01_v[:, :, :, sl0], w1b, ALU.mult)
                    nc.vector.tensor_tensor(tmpx_v[:, :, :, j0 : j0 + L],
                                            R01_v[:, :, :, sl1], wb, ALU.mult)
                    nc.vector.tensor_add(cfx_v[:, :, :, j0 : j0 + L],
                                         cfx_v[:, :, :, j0 : j0 + L],
                                         tmpx_v[:, :, :, j0 : j0 + L])
                tmpy = p0.tile([128, 2, P, P], F32, tag="tmpy")
                shc = [128, 2, P, P]
                nc.vector.tensor_tensor(cfv[:], cfx[:, :, :, 0, :],
                                        cft[:, None, 28:42, None].to_broadcast(shc), ALU.mult)
                nc.vector.tensor_tensor(tmpy[:], cfx[:, :, :, 1, :],
                                        cft[:, None, 14:28, None].to_broadcast(shc), ALU.mult)
                nc.vector.tensor_add(cfv[:], cfv[:], tmpy[:])
                nc.vector.tensor_copy(cfb[:], cfv[:])

                # --- remaining batch-0 operands on DVE
                box_math(0, 0)
                g_build(p0, 0, "b0")

                # --- UC build on PE (phase 0 PSUM; frees loop-phase banks):
                #     UC[qc][q, udy, o] = sum_ci cfb[ci, q] * Wc[ci_hi][ci, udy, o]
                cfv_f = cfb[:].rearrange("p c i j -> p c (i j)")
                for qc in range(2):
                    qn = 128 if qc == 0 else Q1
                    qs = slice(qc * 128, qc * 128 + qn)
                    for udy in range(12):
                        psU = psu0.tile([128, 256], F32, tag="ups",
                                        name=f"ups{qc}_{udy}")
                        for cc in range(2):
                            nc.tensor.matmul(psU[:qn], cfv_f[:, cc, qs],
                                             Wc[:, cc, udy, :],
                                             start=(cc == 0), stop=(cc == 1))
                        nc.scalar.copy(UC[qc][:qn, udy, :], psU[:qn])

                # --- epilogue scalars
                nc.sync.dma_start(epi[:].rearrange("p a b -> p (a b)"),
                                  epi_d.rearrange("p a b -> p (a b)"))
                tmp_e = p0.tile([128, 2], F32, tag="tmp_e")
                eps_t = p0.tile([128, 1], F32, tag="eps_t")
                nc.vector.memset(eps_t[:], 1e-5)
                nc.scalar.activation(tmp_e[:], epi[:, 4, :], AF.Sqrt, bias=eps_t[:], scale=1.0)
                nc.vector.reciprocal(scale_e[:], tmp_e[:])
                nc.vector.tensor_mul(scale_e[:], scale_e[:], epi[:, 1, :])
                nc.vector.tensor_sub(bias_e[:], epi[:, 0, :], epi[:, 3, :])
                nc.vector.tensor_mul(bias_e[:], bias_e[:], scale_e[:])
                nc.vector.tensor_add(bias_e[:], bias_e[:], epi[:, 2, :])

            # ---------- main loop ----------
            with tc.tile_pool(name="loop", bufs=2) as lp, \
                 tc.tile_pool(name="gpool", bufs=2) as gp, \
                 tc.tile_pool(name="psv", bufs=8, space="PSUM") as psv:

                CHUNKS = ([("m", ci, dy) for ci in range(2) for dy in range(3)]
                          + [("c", qc, dy) for qc in range(2) for dy in range(3)])

                def emit_pass(Mt2, XW, GWs, oc):
                    # each lhsT feeds both 4-box halves back-to-back so
                    # LDWEIGHTS is paid once per chunk; u-inner so the same
                    # PSUM tile is revisited at distance 8, not 2
                    for k, (kind, c, dy) in enumerate(CHUNKS):
                        for u in range(4):
                            if kind == "m":
                                qn = 128
                                lhsT = Wm[:, c, u * 3 + dy, oc * 128 : oc * 128 + 128]
                                src = XW[:, c]
                            else:
                                qn = 128 if c == 0 else Q1
                                lhsT = UC[c][:qn, u * 3 + dy, oc * 128 : oc * 128 + 128]
                                src = GWs[c]
                            for h in range(2):
                                ns = slice(h * HB, h * HB + HB)
                                rhs = src[:qn, ns, dy : dy + P, u * 7 : u * 7 + 7]
                                nc.tensor.matmul(Mt2[u][h][:], lhsT, rhs,
                                                 start=(k == 0), stop=(k == 11))

                for b in range(NBATCH):
                    n0 = b * BATCH
                    slot = b % 2
                    XW = XWb[slot]
                    GWs = [GWb[0][slot], GWb[1][slot]]
                    # operands for batch b (XW/GW/box data) were produced during
                    # batch b-1 (batch 0's in phase 0)

                    ost = lp.tile([128, 2, BATCH, PQ], F32, tag="ost")
                    ost_v = ost

                    for oc in range(2):
                        Yt = lp.tile([128, BATCH, P, P], F32, tag="Yt", name=f"Y_{b}_{oc}")
                        Yv = Yt[:].rearrange("p n i j -> p n (i j)")
                        Mt2 = [[psv.tile([128, HB, P, 7], F32, tag="M",
                                         name=f"M_{b}_{oc}_{u}_{h}")
                                for h in range(2)] for u in range(4)]
                        emit_pass(Mt2, XW, GWs, oc)
                        # PSUM -> SBUF on the scalar engine (frees banks fast,
                        # decoupled from DVE bulk work)
                        Mc = lp.tile([128, 4, BATCH, P, 7], F32, tag="Mc",
                                     name=f"Mc_{b}_{oc}")
                        for u in range(4):
                            for h in range(2):
                                nc.scalar.copy(Mc[:, u, h * HB : h * HB + HB],
                                               Mt2[u][h][:])
                        # inverse x-transform: Y[...,0::2] = M0+M1+M2,
                        # Y[...,1::2] = M1-M2-M3; temps on gpsimd, Y on DVE
                        tI = gp.tile([128, 2, BATCH, P, 7], F32, tag="tI",
                                     name=f"tI_{b}_{oc}")
                        nc.vector.tensor_add(tI[:, 0], Mc[:, 0], Mc[:, 1])
                        nc.vector.tensor_sub(tI[:, 1], Mc[:, 1], Mc[:, 2])
                        nc.vector.tensor_add(Yt[:, :, :, 0::2], tI[:, 0], Mc[:, 2])
                        nc.vector.tensor_sub(Yt[:, :, :, 1::2], tI[:, 1], Mc[:, 3])
                        # BN + ReLU + store (split for DMA overlap)
                        for h in range(2):
                            ns = slice(h * HB, h * HB + HB)
                            nc.scalar.activation(
                                ost_v[:, oc, ns], Yv[:, ns],
                                AF.Relu, bias=bias_e[:, oc : oc + 1],
                                scale=scale_e[:, oc : oc + 1],
                            )
                            nc.sync.dma_start(
                                out_v[:, oc, n0 + h * HB : n0 + h * HB + HB],
                                ost_v[:, oc, ns])
                        # next batch's operands, emitted mid-batch so the DVE
                        # produces them while the PE runs this batch's passes
                        if oc == 0 and b + 1 < NBATCH:
                            nslot = (b + 1) % 2
                            box_math(n0 + BATCH, nslot)
                            mstn = lp.tile([128, 2, BATCH, PQ], F32, tag="mst",
                                           name=f"mst{b + 1}")
                            for ch in range(2):
                                nc.sync.dma_start(mstn[:, ch],
                                                  mask_v[:, ch, n0 + BATCH : n0 + 2 * BATCH])
                            xw_build(mstn, nslot)
                            g_build(gp, nslot, "")

    nc.compile()
    return nc


# ---------------------------------------------------------------------------
# host-side sharding / unsharding
# ---------------------------------------------------------------------------

def _prep_in_maps(features, proposal_boxes, mask_features, conv_w, conv_b,
                  bn_gamma, bn_beta, bn_mean, bn_var):
    features = np.asarray(features, dtype=np.float32)
    proposal_boxes = np.asarray(proposal_boxes, dtype=np.float32)
    mask_features = np.asarray(mask_features, dtype=np.float32)
    conv_w = np.asarray(conv_w, dtype=np.float32)
    # weight layout: x-winograd transform Ww[u,dy] = sum_dx G[u,dx] w[.,.,dy,dx]
    # [cout=256, cin=512, 3, 3] -> [cin_par=128, cin_hi=4, u*3+dy (12), cout=256], bf16
    import ml_dtypes
    Gm = np.array([[1, 0, 0], [.5, .5, .5], [.5, -.5, .5], [0, 0, 1]], np.float32)
    wf = conv_w.reshape(256, 4, 128, 3, 3)                     # [o, hi, par, dy, dx]
    ww = np.einsum('ud,ohpyd->phuyo', Gm, wf)                  # [par, hi, u, dy, o]
    wt = np.ascontiguousarray(ww.reshape(128, 4, 12, 256)).astype(ml_dtypes.bfloat16)
    epi = np.stack([np.asarray(x, dtype=np.float32) for x in
                    (conv_b, bn_gamma, bn_beta, bn_mean, bn_var)])  # [5, 256]
    epi = np.ascontiguousarray(epi.reshape(5, 2, 128).transpose(2, 0, 1)).astype(np.float32)
    cp = _consts_p()
    cfc = _consts_f()

    in_maps = []
    for i in range(N_CORES):
        img = i // (N_CORES // 2)
        n0 = (i * NB) % 256
        in_maps.append({
            "features": np.ascontiguousarray(features[img]),
            "boxes": np.ascontiguousarray(proposal_boxes[img, n0 : n0 + NB]),
            "mask": np.ascontiguousarray(mask_features[i * NB : (i + 1) * NB]),
            "wt": wt,
            "epi": epi,
            "consts_p": cp,
            "consts_f": cfc,
        })
    return in_maps


_NC_CACHE = {}


def _get_nc():
    if "nc" not in _NC_CACHE:
        _NC_CACHE["nc"] = build_kernel()
    return _NC_CACHE["nc"]


def _install_ntff_shim():
    """antenv.axon_hooks is missing in this image; shim it so trace=True works."""
    try:
        import antenv
        if hasattr(antenv, "axon_hooks"):
            return
        from trn_agent_boot.trn_boot import _ntff_profile_via_ctypes
        mod = types.ModuleType("antenv.axon_hooks")
        _h = [None]
        mod.set_axon_ntff_profile_hook = lambda h: _h.__setitem__(0, h)
        mod.get_axon_ntff_profile_hook = lambda: _h[0]
        sys.modules["antenv.axon_hooks"] = mod
        antenv.axon_hooks = mod
        mod.set_axon_ntff_profile_hook(_ntff_profile_via_ctypes("/opt/axon/libaxon_pjrt.so"))
    except Exception:
        pass


def run(trace=False, tmpdir=None, **inputs):
    from concourse.bass_utils import run_bass_kernel_spmd

    if trace:
        _install_ntff_shim()
    nc = _get_nc()
    in_maps = _prep_in_maps(**inputs)
    res = run_bass_kernel_spmd(nc, in_maps, core_ids=list(range(N_CORES)),
                               trace=trace, tmpdir=tmpdir)
    out = np.concatenate([np.asarray(res.results[i]["out"]) for i in range(N_CORES)], axis=0)
    return out.astype(np.float32), res


def kernel(**inputs):
    out, _ = run(trace=False, **inputs)
    return out


# revision 14
# speedup vs baseline: 1.2946x; 1.1008x over previous
"""Trainium2 Bass kernel for nn_AddMaskHead (ROI mask head: bilinear pool + concat + conv3x3 + BN + ReLU).

Self-contained: hardcodes shapes B=2, N=256 (512 boxes), C=256, H=96, W=128, P=14.
Shards data-parallel over the 512 boxes across 8 NeuronCores (64 boxes/core; each
core's boxes all come from a single image, so each core only needs its image's
features).

Conv strategy: 1-D Winograd F(2,3) along x. The 3x3 conv is computed as 4
x-positions (u) x 3 row taps (dy) instead of 9 taps x (2x the output columns),
cutting PE streaming 1.5x. The ROI pooling is folded into the conv's crops half
via separable interpolation matrices (my (x) mxw), with the Winograd x-transform
applied analytically to the x-side factor. Each weight load (LDWEIGHTS is not
hidden on trn2) is amortized over two consecutive matmuls (the two 4-box PSUM
halves); the inverse x-transform runs on gpsimd+DVE from SBUF after fast
scalar-engine PSUM evacuation.
"""

import sys, os, types

sys.path.insert(0, "/opt/trn_rl_repo")

import numpy as np
import concourse.bass as bass
import concourse.mybir as mybir
import concourse.tile as tile
from concourse import bacc
from concourse.masks import make_identity

F32 = mybir.dt.float32
BF16 = mybir.dt.bfloat16
I32 = mybir.dt.int32
ALU = mybir.AluOpType
AF = mybir.ActivationFunctionType

N_CORES = 8
NB = 64            # boxes per core
BATCH = 8          # boxes per inner batch
NBATCH = NB // BATCH
HB = 4             # boxes per PSUM half
P = 14             # pooler resolution
C = 256            # channels
H, W = 96, 128     # feature map
PQ = P * P         # 196
Q1 = PQ - 128      # 68 (q-chunk 1 size)


def _axis_static(in_s, out_s=P):
    # mirrors reference._resize_bilinear axis() in exact f32 arithmetic
    s = (np.arange(out_s, dtype=np.float32) + np.float32(0.5)) * np.float32(in_s / out_s) - np.float32(0.5)
    s = np.maximum(s, np.float32(0.0))
    i0 = np.minimum(np.floor(s).astype(np.int32), in_s - 1)
    i1 = np.minimum(i0 + 1, in_s - 1)
    w = (s - i0.astype(np.float32)).astype(np.float32)
    return i0, i1, w


YS0, YS1, WYS = _axis_static(H)
XS0, XS1, WXS = _axis_static(W)


def _consts_p():
    # per-partition constants: [128, 4] = (yv_q0, xv_q0, yv_q1, xv_q1); -1 pads
    arr = np.full((128, 4), -1.0, dtype=np.float32)
    for p in range(128):
        arr[p, 0] = (p // P)
        arr[p, 1] = (p % P)
    for p in range(Q1):
        q = 128 + p
        arr[p, 2] = (q // P)
        arr[p, 3] = (q % P)
    return arr


def _consts_f():
    # free-dim constants (broadcast to all partitions on device):
    # [0:14] jc = arange(14)+0.5; [14:28] wys; [28:42] 1-wys;
    # [48:62] wxs; [62:76] 1-wxs
    arr = np.zeros((1, 80), dtype=np.float32)
    arr[0, 0:14] = np.arange(P, dtype=np.float32) + np.float32(0.5)
    arr[0, 14:28] = WYS
    arr[0, 28:42] = np.float32(1.0) - WYS
    arr[0, 48:62] = WXS
    arr[0, 62:76] = np.float32(1.0) - WXS
    return arr


def build_kernel():
    nc = bacc.Bacc(None)

    featsl = nc.declare_dram_parameter("featsl", [128, 2 * P * 2 * P * 2], F32,
                                       isOutput=False)
    boxes = nc.declare_dram_parameter("boxes", [NB, 4], F32, isOutput=False)
    mask = nc.declare_dram_parameter("mask", [NB, C, P, P], F32, isOutput=False)
    wt_d = nc.declare_dram_parameter("wt", [128, 4, 12, 256], BF16, isOutput=False)
    epi_d = nc.declare_dram_parameter("epi", [128, 5, 2], F32, isOutput=False)
    cp_d = nc.declare_dram_parameter("consts_p", [128, 4], F32, isOutput=False)
    cf_d = nc.declare_dram_parameter("consts_f", [1, 80], F32, isOutput=False)
    out_d = nc.declare_dram_parameter("out", [NB, C, P, P], F32, isOutput=True)

    mask_v = mask.rearrange("n (ch cp) i j -> cp ch n (i j)", cp=128)
    out_v = out_d.rearrange("n (oh op) i j -> op oh n (i j)", op=128)

    RC14 = float(np.float32(1.0) / np.float32(P))

    with tile.TileContext(nc) as tc:
        with tc.tile_pool(name="persist", bufs=1) as pp:
            # ---------- persistent tiles ----------
            # x-winograd-transformed mask-half weights [ci_par, ci_hi, u*3+dy, o]
            Wm = pp.tile([128, 2, 12, 256], BF16, tag="Wm")
            # UC[qc][q, u*3+dy, o] = sum_ci cf[ci, q] * Ww_crop[u,dy][ci, o]
            UC = [pp.tile([128, 12, 256], BF16, tag=f"UC{qc}", name=f"UC{qc}") for qc in range(2)]
            # x-transformed mask features [ci_par, ci_hi, n, r(16 pad rows), u*7+tx]
            XWb = [pp.tile([128, 2, BATCH, 16, 28], BF16, tag=f"xw{i}", name=f"xw{i}")
                   for i in range(2)]
            # x-transformed interp frames per q-chunk (double-buffered)
            GWb = [[pp.tile([128, BATCH, 16, 28], BF16, tag=f"gw{qc}{i}", name=f"gw{qc}{i}")
                    for i in range(2)] for qc in range(2)]
            cpt = pp.tile([128, 4], F32, tag="cpt")
            cft = pp.tile([128, 80], F32, tag="cft")
            epi = pp.tile([128, 5, 2], F32, tag="epi")
            scale_e = pp.tile([128, 2], F32, tag="scale_e")
            bias_e = pp.tile([128, 2], F32, tag="bias_e")
            # per-box interpolation data (ping-pong slots): [128, 2, BATCH, 14] per axis
            Y0 = pp.tile([128, 2, BATCH, P], F32, tag="Y0")
            Y1 = pp.tile([128, 2, BATCH, P], F32, tag="Y1")
            WY = pp.tile([128, 2, BATCH, P], F32, tag="WY")
            OWY = pp.tile([128, 2, BATCH, P], F32, tag="OWY")
            X0 = pp.tile([128, 2, BATCH, P], F32, tag="X0")
            X1 = pp.tile([128, 2, BATCH, P], F32, tag="X1")
            WX = pp.tile([128, 2, BATCH, P], F32, tag="WX")
            OWX = pp.tile([128, 2, BATCH, P], F32, tag="OWX")
            # box-math temps
            bxb = pp.tile([128, NB, 4], F32, tag="bxb")
            abx = pp.tile([128, BATCH, 4], F32, tag="abx")
            bm_i4 = pp.tile([128, BATCH, 4], I32, tag="bm_i4")
            bm_f4 = pp.tile([128, BATCH, 4], F32, tag="bm_f4")
            bm_s = pp.tile([128, BATCH, P], F32, tag="bm_s")
            bm_f = pp.tile([128, BATCH, P], F32, tag="bm_f")
            bm_i = pp.tile([128, BATCH, P], I32, tag="bm_i")
            bm_a = pp.tile([128, BATCH, P], F32, tag="bm_a")
            bm_d = pp.tile([128, BATCH], F32, tag="bm_d")
            bm_n = pp.tile([128, BATCH], F32, tag="bm_n")
            bm_q = pp.tile([128, BATCH], F32, tag="bm_q")
            bm_h = pp.tile([128, BATCH], F32, tag="bm_h")
            cfv = pp.tile([128, 2, P, P], F32, tag="cfv")
            cfb = pp.tile([128, 2, P, P], BF16, tag="cfb")

            jc_b = cft[:, 0:14]

            def g_build(tpool, slot, sfx):
                """x-winograd interp frames GW[q, n, r16, u*7+tx] for one batch
                (ping-pong slot) from box data in `slot`."""
                for qc in range(2):
                    shb = [128, BATCH, P]
                    yv = cpt[:, 2 * qc : 2 * qc + 1, None].to_broadcast(shb)
                    xv = cpt[:, 2 * qc + 1 : 2 * qc + 2, None].to_broadcast(shb)
                    my = tpool.tile([128, BATCH, P], F32, tag=f"my{qc}{sfx}", name=f"my{qc}{sfx}")
                    mx = tpool.tile([128, BATCH, P], F32, tag=f"mx{qc}{sfx}", name=f"mx{qc}{sfx}")
                    cmp = tpool.tile([128, BATCH, P], F32, tag=f"cmp{qc}{sfx}", name=f"cmp{qc}{sfx}")
                    mxw = tpool.tile([128, BATCH, 28], F32, tag=f"mxw{qc}{sfx}", name=f"mxw{qc}{sfx}")
                    bsl = (slice(None), slot, slice(None), slice(None))
                    nc.vector.tensor_tensor(my[:], Y0[bsl], yv, ALU.is_equal)
                    nc.vector.tensor_mul(my[:], my[:], OWY[bsl])
                    nc.vector.tensor_tensor(cmp[:], Y1[bsl], yv, ALU.is_equal)
                    nc.vector.tensor_mul(cmp[:], cmp[:], WY[bsl])
                    nc.vector.tensor_add(my[:], my[:], cmp[:])
                    nc.vector.tensor_tensor(mx[:], X0[bsl], xv, ALU.is_equal)
                    nc.vector.tensor_mul(mx[:], mx[:], OWX[bsl])
                    nc.vector.tensor_tensor(cmp[:], X1[bsl], xv, ALU.is_equal)
                    nc.vector.tensor_mul(cmp[:], cmp[:], WX[bsl])
                    nc.vector.tensor_add(mx[:], mx[:], cmp[:])
                    # x-winograd transform of mx (pad col c in 1..14 <-> mx j=c-1)
                    nc.vector.tensor_sub(mxw[:, :, 1:7], mx[:, :, 1:12:2], mx[:, :, 3:14:2])
                    nc.vector.tensor_scalar_mul(mxw[:, :, 0:1], mx[:, :, 1:2], -1.0)
                    nc.vector.tensor_add(mxw[:, :, 7:14], mx[:, :, 0:14:2], mx[:, :, 1:14:2])
                    nc.vector.tensor_sub(mxw[:, :, 14:21], mx[:, :, 1:14:2], mx[:, :, 0:14:2])
                    nc.vector.tensor_sub(mxw[:, :, 21:27], mx[:, :, 0:12:2], mx[:, :, 2:14:2])
                    nc.vector.tensor_copy(mxw[:, :, 27:28], mx[:, :, 12:13])
                    # GW[:, n, 1:15, :] = my (x) mxw  (rows 0,15 stay zero)
                    GW = GWb[qc][slot]
                    shg = [128, BATCH, P, 28]
                    nc.vector.tensor_tensor(GW[:, :, 1:15, :],
                                            my[:, :, :, None].to_broadcast(shg),
                                            mxw[:, :, None, :].to_broadcast(shg), ALU.mult)

            def xw_build(mst, slot):
                """x-winograd transform of mask features into XWb[slot]."""
                XW = XWb[slot]
                v = mst[:].rearrange("p c n (i j) -> p c n i j", j=P)
                for ci in range(2):
                    xw = XW[:, ci, :, 1:15, :]
                    vi = v[:, ci]
                    nc.vector.tensor_add(xw[:, :, :, 7:14], vi[:, :, :, 0:14:2], vi[:, :, :, 1:14:2])
                    nc.vector.tensor_sub(xw[:, :, :, 14:21], vi[:, :, :, 1:14:2], vi[:, :, :, 0:14:2])
                    nc.vector.tensor_sub(xw[:, :, :, 1:7], vi[:, :, :, 1:12:2], vi[:, :, :, 3:14:2])
                    nc.vector.tensor_scalar_mul(xw[:, :, :, 0:1], vi[:, :, :, 1:2], -1.0)
                    nc.vector.tensor_sub(xw[:, :, :, 21:27], vi[:, :, :, 0:12:2], vi[:, :, :, 2:14:2])
                    nc.vector.tensor_copy(xw[:, :, :, 27:28], vi[:, :, :, 12:13])

            def box_math(n0, slot):
                """fill per-axis index/weight arrays for boxes [n0, n0+BATCH) into slot"""
                nn = BATCH
                ns = slice(n0, n0 + nn)
                t, fr, ti = abx[:], bm_f4[:], bm_i4[:]
                nc.vector.tensor_scalar_mul(t[:], bxb[:, ns], 0.125)
                nc.vector.tensor_copy(ti[:], t[:])
                nc.vector.tensor_copy(fr[:], ti[:])
                nc.vector.tensor_tensor(ti[:].bitcast(F32), fr[:], t[:], ALU.is_gt)
                nc.vector.tensor_sub(t[:], fr[:], ti[:].bitcast(F32))
                d, nlt, beq, adj = bm_d[:], bm_n[:], bm_q[:], bm_h[:]
                for ax in range(2):  # 0: x (cols 0,2), 1: y (cols 1,3)
                    a_io, b_io = t[:, :, ax], t[:, :, 2 + ax]
                    nc.vector.tensor_sub(d[:], b_io, a_io)
                    nc.vector.tensor_scalar(nlt[:], d[:], 1.0, None, ALU.is_lt)
                    nc.vector.tensor_scalar(beq[:], b_io, float(P), None, ALU.is_equal)
                    nc.vector.tensor_mul(adj[:], nlt[:], beq[:])
                    nc.vector.tensor_sub(a_io, a_io, adj[:])
                    nc.vector.tensor_add(b_io, b_io, nlt[:])
                    nc.vector.tensor_sub(b_io, b_io, adj[:])
                nwid, him1 = bm_d[:], bm_n[:]
                s, frs, si, i0c = bm_s[:], bm_f[:], bm_i[:], bm_a[:]
                sh3 = [128, nn, P]
                for ax, (I0, I1, Wf, OWf) in enumerate(
                    [(X0, X1, WX, OWX), (Y0, Y1, WY, OWY)]
                ):
                    ssl = (slice(None), slot)
                    lo_b = t[:, :, ax][:, :, None].to_broadcast(sh3)
                    nc.vector.tensor_sub(nwid[:], t[:, :, 2 + ax], t[:, :, ax])
                    nc.vector.tensor_scalar_sub(him1[:], nwid[:], 1.0)
                    h_b = him1[:, :, None].to_broadcast(sh3)
                    nc.vector.tensor_tensor(s[:], nwid[:, :, None].to_broadcast(sh3),
                                            jc_b[:, None, :].to_broadcast(sh3), ALU.mult)
                    nc.vector.tensor_scalar(s[:], s[:], RC14, -0.5, ALU.mult, ALU.add)
                    nc.vector.tensor_scalar(s[:], s[:], 0.0, None, ALU.max)
                    nc.vector.tensor_copy(si[:], s[:])
                    nc.vector.tensor_copy(frs[:], si[:])
                    nc.vector.tensor_tensor(si[:].bitcast(F32), frs[:], s[:], ALU.is_gt)
                    nc.vector.tensor_sub(i0c[:], frs[:], si[:].bitcast(F32))
                    nc.vector.tensor_tensor(i0c[:], i0c[:], h_b, ALU.min)
                    nc.vector.tensor_sub(Wf[ssl], s[:], i0c[:])
                    nc.vector.tensor_scalar(OWf[ssl], Wf[ssl], -1.0, 1.0, ALU.mult, ALU.add)
                    nc.vector.tensor_add(I0[ssl], i0c[:], lo_b)
                    nc.vector.tensor_scalar_add(i0c[:], i0c[:], 1.0)
                    nc.vector.tensor_tensor(i0c[:], i0c[:], h_b, ALU.min)
                    nc.vector.tensor_add(I1[ssl], i0c[:], lo_b)

            # ---------- phase 0 ----------
            with tc.tile_pool(name="ph0", bufs=1) as p0, \
                 tc.tile_pool(name="ps0", bufs=1, space="PSUM") as ps0, \
                 tc.tile_pool(name="psu", bufs=6, space="PSUM") as psu0:

                # --- tiny gating DMAs first (bx1 gates the broadcasts)
                ones1 = p0.tile([1, 128], F32, tag="ones1")
                nc.gpsimd.memset(ones1[:], 1.0)
                bx1 = p0.tile([1, NB * 4], F32, tag="bx1")
                nc.sync.dma_start(bx1[:], boxes.rearrange("n f -> (n f)")[None, :])
                cf1 = p0.tile([1, 80], F32, tag="cf1")
                nc.sync.dma_start(cf1[:], cf_d[:])
                nc.sync.dma_start(cpt[:], cp_d[:])

                # --- host pre-sliced feature rows/cols (28x28 of 96x128):
                #     gates the cf -> UC chain; 4 descriptors across queues
                R01c = p0.tile([128, 2, P, 2, P, 2], F32, tag="R01c")
                r01_f = R01c[:].rearrange("p c i r j s -> p (c i r j s)")
                QS = P * 2 * P * 2 // 2  # 392: half of one ch chunk
                for d in range(4):
                    nc.sync.dma_start(r01_f[:, d * QS : (d + 1) * QS],
                                      featsl[:, d * QS : (d + 1) * QS])

                # --- mask batch 0 (gates xw_build(0)), weights
                mst0 = p0.tile([128, 2, BATCH, PQ], F32, tag="mst0")
                nc.sync.dma_start(mst0[:, 0], mask_v[:, 0, 0:BATCH])
                nc.sync.dma_start(mst0[:, 1], mask_v[:, 1, 0:BATCH])
                Wc = p0.tile([128, 2, 12, 256], BF16, tag="Wc")
                nc.sync.dma_start(Wc[:].rearrange("p a b c -> p (a b c)"),
                                  wt_d[:, 2:4].rearrange("p a b c -> p (a b c)"))
                nc.sync.dma_start(Wm[:].rearrange("p a b c -> p (a b c)"),
                                  wt_d[:, 0:2].rearrange("p a b c -> p (a b c)"))

                # --- broadcasts via K=1 matmul with ones (PE is idle here)
                psb = ps0.tile([128, 256], F32, tag="psb")
                nc.tensor.matmul(psb[:], ones1[:], bx1[:])
                nc.scalar.copy(bxb[:].rearrange("p n f -> p (n f)"), psb[:])
                psf = ps0.tile([128, 80], F32, tag="psf")
                nc.tensor.matmul(psf[:], ones1[:], cf1[:])
                nc.scalar.copy(cft[:], psf[:])

                # --- one-time zeroing of pad rows r=0,15 (gpsimd; interiors are
                #     rewritten every batch)
                for i in range(2):
                    nc.gpsimd.memset(XWb[i][:, :, :, 0, :], 0.0)
                    nc.gpsimd.memset(XWb[i][:, :, :, 15, :], 0.0)
                for qc in range(2):
                    for i in range(2):
                        nc.gpsimd.memset(GWb[qc][i][:, :, 0, :], 0.0)
                        nc.gpsimd.memset(GWb[qc][i][:, :, 15, :], 0.0)
                # UC[1] zeroed first so its pad rows (partitions 68..127)
                # are 0 and qc1 matmuls can run K=128; the UC build then
                # overwrites partitions 0..67
                nc.gpsimd.memset(UC[1][:], 0.0)

                # --- concat-features (cf) first on DVE (gates UC; R01c
                #     lands before mst0): x-lerp on the pre-sliced col
                #     pairs, weights from free-dim consts; then y-lerp
                cfx = p0.tile([128, 2, P, 2, P], F32, tag="cfx")  # (ch, i, r, j)
                tmpx = p0.tile([128, 2, P, 2, P], F32, tag="tmpx")
                cfx_v = cfx[:].rearrange("p c i r j -> p c (i r) j")
                tmpx_v = tmpx[:].rearrange("p c i r j -> p c (i r) j")
                R01_m = R01c[:].rearrange("p c i r j s -> p c (i r) (j s)")
                shL = [128, 2, 2 * P, P]
                w1b = cft[:, None, None, 62:76].to_broadcast(shL)
                wb = cft[:, None, None, 48:62].to_broadcast(shL)
                nc.vector.tensor_tensor(cfx_v[:], R01_m[:, :, :, 0::2], w1b, ALU.mult)
                nc.vector.tensor_tensor(tmpx_v[:], R01_m[:, :, :, 1::2], wb, ALU.mult)
                nc.vector.tensor_add(cfx_v[:], cfx_v[:], tmpx_v[:])
                tmpy = p0.tile([128, 2, P, P], F32, tag="tmpy")
                shc = [128, 2, P, P]
                nc.vector.tensor_tensor(cfv[:], cfx[:, :, :, 0, :],
                                        cft[:, None, 28:42, None].to_broadcast(shc), ALU.mult)
                nc.vector.tensor_tensor(tmpy[:], cfx[:, :, :, 1, :],
                                        cft[:, None, 14:28, None].to_broadcast(shc), ALU.mult)
                nc.vector.tensor_add(cfv[:], cfv[:], tmpy[:])
                nc.vector.tensor_copy(cfb[:], cfv[:])

                # --- remaining batch-0 operands on DVE
                xw_build(mst0, 0)
                box_math(0, 0)
                g_build(p0, 0, "b0")

                # --- UC build on PE (phase 0 PSUM; frees loop-phase banks):
                #     UC[qc][q, udy, o] = sum_ci cfb[ci, q] * Wc[ci_hi][ci, udy, o]
                cfv_f = cfb[:].rearrange("p c i j -> p c (i j)")
                for qc in range(2):
                    qn = 128 if qc == 0 else Q1
                    qs = slice(qc * 128, qc * 128 + qn)
                    for wv in range(2):  # waves of 6 udy: lhsT reused 6x per cc
                        tiles = [psu0.tile([128, 256], F32, tag="ups",
                                           name=f"ups{qc}_{wv}_{j}")
                                 for j in range(6)]
                        for cc in range(2):
                            for j in range(6):
                                nc.tensor.matmul(tiles[j][:qn], cfv_f[:, cc, qs],
                                                 Wc[:, cc, wv * 6 + j, :],
                                                 start=(cc == 0), stop=(cc == 1))
                        for j in range(6):
                            nc.scalar.copy(UC[qc][:qn, wv * 6 + j, :], tiles[j][:qn])

                # --- epilogue scalars
                nc.sync.dma_start(epi[:].rearrange("p a b -> p (a b)"),
                                  epi_d.rearrange("p a b -> p (a b)"))
                tmp_e = p0.tile([128, 2], F32, tag="tmp_e")
                eps_t = p0.tile([128, 1], F32, tag="eps_t")
                nc.vector.memset(eps_t[:], 1e-5)
                nc.scalar.activation(tmp_e[:], epi[:, 4, :], AF.Sqrt, bias=eps_t[:], scale=1.0)
                nc.vector.reciprocal(scale_e[:], tmp_e[:])
                nc.vector.tensor_mul(scale_e[:], scale_e[:], epi[:, 1, :])
                nc.vector.tensor_sub(bias_e[:], epi[:, 0, :], epi[:, 3, :])
                nc.vector.tensor_mul(bias_e[:], bias_e[:], scale_e[:])
                nc.vector.tensor_add(bias_e[:], bias_e[:], epi[:, 2, :])

            # ---------- main loop ----------
            with tc.tile_pool(name="loop", bufs=2) as lp, \
                 tc.tile_pool(name="gpool", bufs=2) as gp, \
                 tc.tile_pool(name="psv", bufs=8, space="PSUM") as psv:

                # dy=1 first in each half so the start=True matmul covers
                # every dest element (dy=0/2 skip a known-zero pad row)
                CHUNKS = ([("m", ci, dy) for ci in range(2) for dy in (1, 0, 2)]
                          + [("c", qc, dy) for qc in range(2) for dy in (1, 0, 2)])

                def emit_pass(Mt2, XW, GWs, oc):
                    # each lhsT feeds both 4-box halves back-to-back so
                    # LDWEIGHTS is paid once per chunk; u-inner so the same
                    # PSUM tile is revisited at distance 8, not 2
                    for k, (kind, c, dy) in enumerate(CHUNKS):
                        for u in range(4):
                            if kind == "m":
                                lhsT = Wm[:, c, u * 3 + dy, oc * 128 : oc * 128 + 128]
                                src = XW[:, c]
                            else:
                                # K padded to 128 for qc1 too: GW partitions
                                # 68..127 are zero and UC[1] pad rows are
                                # memset, keeping every matmul at K=128 (mixed
                                # K was observed to disable fast weight load)
                                lhsT = UC[c][:, u * 3 + dy, oc * 128 : oc * 128 + 128]
                                src = GWs[c]
                            # pad row r=0 (dy=0) / r=15 (dy=2) is zero:
                            # skip that output row, streaming 364 cols not 392
                            if dy == 0:
                                rlo, nr, dsl = 1, P - 1, slice(1, P)
                            elif dy == 2:
                                rlo, nr, dsl = 2, P - 1, slice(0, P - 1)
                            else:
                                rlo, nr, dsl = 1, P, slice(0, P)
                            for h in range(2):
                                ns = slice(h * HB, h * HB + HB)
                                rhs = src[:, ns, rlo : rlo + nr, u * 7 : u * 7 + 7]
                                nc.tensor.matmul(Mt2[u][h][:, :, dsl, :], lhsT, rhs,
                                                 start=(k == 0), stop=(k == 11))

                for b in range(NBATCH):
                    n0 = b * BATCH
                    slot = b % 2
                    XW = XWb[slot]
                    GWs = [GWb[0][slot], GWb[1][slot]]
                    # operands for batch b (XW/GW/box data) were produced during
                    # batch b-1 (batch 0's in phase 0)

                    ost = lp.tile([128, 2, BATCH, PQ], F32, tag="ost")
                    ost_v = ost

                    for oc in range(2):
                        Yt = lp.tile([128, BATCH, P, P], F32, tag="Yt", name=f"Y_{b}_{oc}")
                        Yv = Yt[:].rearrange("p n i j -> p n (i j)")
                        Mt2 = [[psv.tile([128, HB, P, 7], F32, tag="M",
                                         name=f"M_{b}_{oc}_{u}_{h}")
                                for h in range(2)] for u in range(4)]
                        emit_pass(Mt2, XW, GWs, oc)
                        tI = gp.tile([128, 2, BATCH, P, 7], F32, tag="tI",
                                     name=f"tI_{b}_{oc}")
                        if b == NBATCH - 1 and oc == 1:
                            # final pass: banks need no recycling, so run the
                            # inverse straight off PSUM on DVE (one PSUM
                            # operand per op; only M1 staged) - shorter tail
                            McL = lp.tile([128, BATCH, P, 7], F32, tag="McL")
                            for h in range(2):
                                hs = slice(h * HB, h * HB + HB)
                                nc.scalar.copy(McL[:, hs], Mt2[1][h][:])
                            for h in range(2):
                                hs = slice(h * HB, h * HB + HB)
                                nc.vector.tensor_add(tI[:, 0, hs], Mt2[0][h][:],
                                                     McL[:, hs])
                                nc.vector.tensor_add(Yt[:, hs, :, 0::2],
                                                     tI[:, 0, hs], Mt2[2][h][:])
                                nc.vector.tensor_sub(tI[:, 1, hs], McL[:, hs],
                                                     Mt2[2][h][:])
                                nc.vector.tensor_sub(Yt[:, hs, :, 1::2],
                                                     tI[:, 1, hs], Mt2[3][h][:])
                        else:
                            # PSUM -> SBUF on the scalar engine (frees banks
                            # fast, decoupled from DVE bulk work)
                            Mc = lp.tile([128, 4, BATCH, P, 7], F32, tag="Mc",
                                         name=f"Mc_{b}_{oc}")
                            for u in range(4):
                                for h in range(2):
                                    nc.scalar.copy(Mc[:, u, h * HB : h * HB + HB],
                                                   Mt2[u][h][:])
                            # inverse x-transform: Y[...,0::2] = M0+M1+M2,
                            # Y[...,1::2] = M1-M2-M3
                            nc.vector.tensor_add(tI[:, 0], Mc[:, 0], Mc[:, 1])
                            nc.vector.tensor_sub(tI[:, 1], Mc[:, 1], Mc[:, 2])
                            nc.vector.tensor_add(Yt[:, :, :, 0::2], tI[:, 0], Mc[:, 2])
                            nc.vector.tensor_sub(Yt[:, :, :, 1::2], tI[:, 1], Mc[:, 3])
                        # BN + ReLU + store (split for DMA overlap; finer
                        # split on the last pass to shorten the tail)
                        nsplit = 4 if (b == NBATCH - 1 and oc == 1) else 2
                        nb_s = BATCH // nsplit
                        for h in range(nsplit):
                            ns = slice(h * nb_s, h * nb_s + nb_s)
                            nc.scalar.activation(
                                ost_v[:, oc, ns], Yv[:, ns],
                                AF.Relu, bias=bias_e[:, oc : oc + 1],
                                scale=scale_e[:, oc : oc + 1],
                            )
                            nc.sync.dma_start(
                                out_v[:, oc, n0 + h * nb_s : n0 + (h + 1) * nb_s],
                                ost_v[:, oc, ns])
                        # next batch's operands, emitted mid-batch so the DVE
                        # produces them while the PE runs this batch's passes
                        if oc == 0 and b + 1 < NBATCH:
                            nslot = (b + 1) % 2
                            box_math(n0 + BATCH, nslot)
                            mstn = lp.tile([128, 2, BATCH, PQ], F32, tag="mst",
                                           name=f"mst{b + 1}")
                            for ch in range(2):
                                nc.sync.dma_start(mstn[:, ch],
                                                  mask_v[:, ch, n0 + BATCH : n0 + 2 * BATCH])
                            xw_build(mstn, nslot)
                            g_build(gp, nslot, "")

    nc.compile()
    return nc


# ---------------------------------------------------------------------------
# host-side sharding / unsharding
# ---------------------------------------------------------------------------

def _prep_in_maps(features, proposal_boxes, mask_features, conv_w, conv_b,
                  bn_gamma, bn_beta, bn_mean, bn_var):
    features = np.asarray(features, dtype=np.float32)
    proposal_boxes = np.asarray(proposal_boxes, dtype=np.float32)
    mask_features = np.asarray(mask_features, dtype=np.float32)
    conv_w = np.asarray(conv_w, dtype=np.float32)
    # weight layout: x-winograd transform Ww[u,dy] = sum_dx G[u,dx] w[.,.,dy,dx]
    # [cout=256, cin=512, 3, 3] -> [cin_par=128, cin_hi=4, u*3+dy (12), cout=256], bf16
    import ml_dtypes
    Gm = np.array([[1, 0, 0], [.5, .5, .5], [.5, -.5, .5], [0, 0, 1]], np.float32)
    wf = conv_w.reshape(256, 4, 128, 3, 3)                     # [o, hi, par, dy, dx]
    ww = np.einsum('ud,ohpyd->phuyo', Gm, wf)                  # [par, hi, u, dy, o]
    wt = np.ascontiguousarray(ww.reshape(128, 4, 12, 256)).astype(ml_dtypes.bfloat16)
    epi = np.stack([np.asarray(x, dtype=np.float32) for x in
                    (conv_b, bn_gamma, bn_beta, bn_mean, bn_var)])  # [5, 256]
    epi = np.ascontiguousarray(epi.reshape(5, 2, 128).transpose(2, 0, 1)).astype(np.float32)
    cp = _consts_p()
    cfc = _consts_f()

    in_maps = []
    for i in range(N_CORES):
        img = i // (N_CORES // 2)
        n0 = (i * NB) % 256
        fimg = features[img]
        fsl = fimg[:, np.stack([YS0, YS1], axis=1), :]          # [256, 14, 2, W]
        fsl = fsl[:, :, :, np.stack([XS0, XS1], axis=1)]        # [256, 14, 2, 14, 2]
        fsl = np.ascontiguousarray(
            fsl.reshape(2, 128, P * 2 * P * 2).transpose(1, 0, 2).reshape(128, -1))
        in_maps.append({
            "featsl": fsl,
            "boxes": np.ascontiguousarray(proposal_boxes[img, n0 : n0 + NB]),
            "mask": np.ascontiguousarray(mask_features[i * NB : (i + 1) * NB]),
            "wt": wt,
            "epi": epi,
            "consts_p": cp,
            "consts_f": cfc,
        })
    return in_maps


_NC_CACHE = {}


def _get_nc():
    if "nc" not in _NC_CACHE:
        _NC_CACHE["nc"] = build_kernel()
    return _NC_CACHE["nc"]


def _install_ntff_shim():
    """antenv.axon_hooks is missing in this image; shim it so trace=True works."""
    try:
        import antenv
        if hasattr(antenv, "axon_hooks"):
            return
        from trn_agent_boot.trn_boot import _ntff_profile_via_ctypes
        mod = types.ModuleType("antenv.axon_hooks")
        _h = [None]
        mod.set_axon_ntff_profile_hook = lambda h: _h.__setitem__(0, h)
        mod.get_axon_ntff_profile_hook = lambda: _h[0]
        sys.modules["antenv.axon_hooks"] = mod
        antenv.axon_hooks = mod
        mod.set_axon_ntff_profile_hook(_ntff_profile_via_ctypes("/opt/axon/libaxon_pjrt.so"))
    except Exception:
        pass


def run(trace=False, tmpdir=None, **inputs):
    from concourse.bass_utils import run_bass_kernel_spmd

    if trace:
        _install_ntff_shim()
    nc = _get_nc()
    in_maps = _prep_in_maps(**inputs)
    res = run_bass_kernel_spmd(nc, in_maps, core_ids=list(range(N_CORES)),
                               trace=trace, tmpdir=tmpdir)
    out = np.concatenate([np.asarray(res.results[i]["out"]) for i in range(N_CORES)], axis=0)
    return out.astype(np.float32), res


def kernel(**inputs):
    out, _ = run(trace=False, **inputs)
    return out
